# revision 16
# baseline (speedup 1.0000x reference)
"""Causal attention kernel for Trainium2, 8 NeuronCores — depth-split fp8.

Problem: x[4,2048,2048] @ Wq/Wk/Wv[2048,2048] -> causal softmax attention.

Sharding (as baseline): 2 cores per batch; each core owns 1024 query rows as
global 512-row chunks {0,3} (even cores) / {1,2} (odd cores). Pairwise
AllGather assembles full K^T / V per batch. Gathered key-block positions are
[chunk0, chunk3, chunk1, chunk2] on every core; query slot 0 (c_lo) attends
positions {0-3, 8-11}, slot 1 (c_hi) all 16; causality via mask tensors.

Depth-split fp8 (new): rows in chunks 2,3 (slot 1, >=1025 visible keys) have
diffuse softmax, so fp8-e4m3 noise (~4%/tensor) attenuates to <1% there:
  - slot-1 Q/K/V projections computed in fp8 DoubleRow (2 fp8/PE cell,
    256-deep contraction per pass), weights host-scaled x64, staged back
    at 1/64.
  - slot-1 scores fp8 DoubleRow; exp shifted by -2 so unnormalized fp8
    probs stay < 240 (TRN e4m3 max); shift cancels in normalization.
  - slot-1 AV in fp8 DoubleRow (adjacent key-block pairs; odd-length
    causal runs rounded up — the padded block's probs are mask-zeroed).
  - slot-0 (chunks 0,1) stays bf16 end-to-end: shallow rows concentrate
    softmax mass and pass v through nearly verbatim (fp8 would blow the
    2e-2 gate; sim: bf16 4.9e-3, full-fp8 4.5e-2, this split 1.33e-2).
c_lo K/V are computed in bf16 and dual-staged (bf16 for slot-0 + fp8 copy
for slot-1); c_hi rows never need bf16 x at all.

dtypes: bf16 matmuls elsewhere; all PE accumulation fp32.
"""

import math

import numpy as np
import ml_dtypes

import concourse.bass as bass
import concourse.mybir as mybir
import concourse.tile as tile
from concourse import bacc
from concourse.bass import ds, ts
from concourse.bass_utils import run_bass_kernel_spmd

B, S, D = 4, 2048, 2048
P = 128
DC = D // P          # 16 contraction chunks
SB = S // P          # 16 key blocks
QROWS = 1024         # query rows per core
NCORES = 8
INV_SQRT_D = 1.0 / math.sqrt(D)
WS = 64.0            # host scale on fp8 weights (keeps W8 in e4m3 normal range)
ESHIFT = -2.0        # exp shift: unnormalized fp8 probs < 240

# gathered key-block position -> true 512-chunk (pair-rank order, all cores)
POS2TRUE = [0, 3, 1, 2]
SLOT0_POS = [0, 1, 2, 3, 8, 9, 10, 11]   # slot-0's (bf16) key positions
S0IDX = {pos: j for j, pos in enumerate(SLOT0_POS)}
PAIRS = [[0, 1], [2, 3], [4, 5], [6, 7]]

F32 = mybir.dt.float32
BF16 = mybir.dt.bfloat16
F8 = mybir.dt.float8e4
DR = mybir.MatmulPerfMode.DoubleRow
Exp = mybir.ActivationFunctionType.Exp
Copy = mybir.ActivationFunctionType.Copy

_CACHED_NC = None


def build_nc():
    global _CACHED_NC
    if _CACHED_NC is not None:
        return _CACHED_NC
    nc = bacc.Bacc(trn_type="TRN2", target_bir_lowering=False, debug=False,
                   num_devices=NCORES)

    xtb_d = nc.dram_tensor("xtb", [D, 512], BF16, kind="ExternalInput")
    xt8_d = nc.dram_tensor("xt8", [D, 512], F8, kind="ExternalInput")
    wqb_d = nc.dram_tensor("wqb", [DC, P, DC, P], BF16, kind="ExternalInput")
    wq8_d = nc.dram_tensor("wq8", [DC, P, DC, P], F8, kind="ExternalInput")
    wkb_d = nc.dram_tensor("wkb", [DC, P, DC, P], BF16, kind="ExternalInput")
    wk8_d = nc.dram_tensor("wk8", [DC, P, DC, P], F8, kind="ExternalInput")
    wvb_d = nc.dram_tensor("wvb", [4, 2, P, 8, 512], BF16, kind="ExternalInput")
    wv8_d = nc.dram_tensor("wv8", [4, 2, P, 8, 512], F8, kind="ExternalInput")
    mkb_d = nc.dram_tensor("maskb", [P, 8, 512], BF16, kind="ExternalInput")
    mk8_d = nc.dram_tensor("mask8", [P, 16, 512], F8, kind="ExternalInput")
    out_d = nc.dram_tensor("out", [QROWS, D], F32, kind="ExternalOutput")

    with tile.TileContext(nc) as tc:
        with (
            tc.tile_pool(name="dram", bufs=1, space="DRAM") as dpool,
            tc.tile_pool(name="ps", bufs=8, space="PSUM") as ps_all,
        ):
            qTb = dpool.tile([P, DC, 512], BF16, tag="qTb")
            qT8 = dpool.tile([P, DC, 512], F8, tag="qT8")
            kTb_own = dpool.tile([4, P, DC, P], BF16, tag="kTbo")
            kT8_own = [dpool.tile([4, P, DC, P], F8, tag=f"kT8o{s}",
                                  name=f"kT8o{s}") for s in range(2)]
            kgb = dpool.tile([2, 4, P, DC, P], BF16, tag="kgb")
            kg8 = [dpool.tile([2, 4, P, DC, P], F8, tag=f"kg8{s}",
                              name=f"kg8{s}") for s in range(2)]
            vvb_own = [dpool.tile([2, P, D], BF16, tag=f"vvbo{g}",
                                  name=f"vvbo{g}") for g in range(2)]
            vv8_own = [dpool.tile([2, P, D], F8, tag=f"vv8o{g}",
                                  name=f"vv8o{g}") for g in range(4)]
            vgb = [dpool.tile([2, 2, P, D], BF16, tag=f"vgb{g}",
                              name=f"vgb{g}") for g in range(2)]
            vg8 = [dpool.tile([2, 2, P, D], F8, tag=f"vg8{g}",
                              name=f"vg8{g}") for g in range(4)]

            # ---------------- phase 1: projections ----------------
            with (
                tc.tile_pool(name="xt", bufs=1) as xt_pool,
                tc.tile_pool(name="wbf", bufs=12) as wbf_pool,
                tc.tile_pool(name="w8", bufs=16) as w8_pool,
                tc.tile_pool(name="wv", bufs=6) as wv_pool,
                tc.tile_pool(name="wv8", bufs=6) as wv8_pool,
                tc.tile_pool(name="st", bufs=10) as st_pool,
                tc.tile_pool(name="st8", bufs=10) as st8_pool,
                tc.tile_pool(name="sv", bufs=6) as sv_pool,
                tc.tile_pool(name="sv8", bufs=6) as sv8_pool,
            ):
                def load_w(pool, dram, m, dt, name):
                    wt = pool.tile([P, DC, P], dt, tag="w", name=name)
                    nc.sync.dma_start(wt[:, :8, :], dram.ap()[m][:, :8, :])
                    nc.sync.dma_start(wt[:, 8:, :], dram.ap()[m][:, 8:, :])
                    return wt

                wkb = [load_w(wbf_pool, wkb_d, 0, BF16, "wkb0")]
                xtb = xt_pool.tile([P, DC, 512], BF16, tag="xtb")
                for dc in range(DC):
                    nc.sync.dma_start(xtb[:, dc, :], xtb_d.ap()[ds(dc * P, P), :])
                wkb += [load_w(wbf_pool, wkb_d, m, BF16, f"wkb{m}")
                        for m in range(1, 8)]
                xt8 = xt_pool.tile([P, DC, 512], F8, tag="xt8")
                for dc in range(DC):
                    nc.sync.dma_start(xt8[:, dc, :], xt8_d.ap()[ds(dc * P, P), :])
                wkb += [load_w(wbf_pool, wkb_d, m, BF16, f"wkb{m}")
                        for m in range(8, DC)]
                wk8 = [load_w(w8_pool, wk8_d, m, F8, f"wk8{m}")
                       for m in range(DC)]

                # --- K c_lo rows: bf16, dual-staged (bf16 + fp8)
                for m in range(DC):
                    ps = ps_all.tile([P, 512], F32, tag="ps")
                    for dc in range(DC):
                        nc.tensor.matmul(
                            ps[:], lhsT=wkb[m][:, dc, :], rhs=xtb[:, dc, :],
                            start=(dc == 0), stop=(dc == DC - 1),
                        )
                    stb = st_pool.tile([P, 512], BF16, tag="st")
                    nc.scalar.copy(stb[:], ps[:])
                    st8 = st8_pool.tile([P, 512], F8, tag="st8")
                    nc.vector.tensor_copy(st8[:], ps[:])
                    for j in range(4):
                        nc.scalar.dma_start(kTb_own[j, :, m, :], stb[:, ts(j, P)])
                        nc.scalar.dma_start(kT8_own[0][j, :, m, :],
                                            st8[:, ts(j, P)])
                nc.gpsimd.collective_compute(
                    "AllGather", mybir.AluOpType.bypass, replica_groups=PAIRS,
                    ins=[kTb_own.opt()], outs=[kgb.opt()])
                nc.gpsimd.collective_compute(
                    "AllGather", mybir.AluOpType.bypass, replica_groups=PAIRS,
                    ins=[kT8_own[0].opt()], outs=[kg8[0].opt()])

                # --- K c_hi rows: fp8 DoubleRow
                for m in range(DC):
                    ps = ps_all.tile([P, 512], F32, tag="ps")
                    for t in range(8):
                        nc.tensor.matmul(
                            ps[:], lhsT=wk8[m][:, ds(2 * t, 2), :],
                            rhs=xt8[:, ds(2 * t, 2), :],
                            start=(t == 0), stop=(t == 7), perf_mode=DR,
                        )
                    st8 = st8_pool.tile([P, 512], F8, tag="st8")
                    nc.scalar.activation(st8[:], ps[:], Copy, scale=1.0 / WS)
                    for j in range(4):
                        nc.scalar.dma_start(kT8_own[1][j, :, m, :],
                                            st8[:, ts(j, P)])
                nc.gpsimd.collective_compute(
                    "AllGather", mybir.AluOpType.bypass, replica_groups=PAIRS,
                    ins=[kT8_own[1].opt()], outs=[kg8[1].opt()])

                # --- V: n-outer so only 2 wv tile pairs are resident
                def load_wv(pool, dram, n, hb, dt, name):
                    wvt = pool.tile([P, 8, 512], dt, tag="wv", name=name)
                    nc.sync.dma_start(wvt[:, :4, :], dram.ap()[n, hb][:, :4, :])
                    nc.sync.dma_start(wvt[:, 4:, :], dram.ap()[n, hb][:, 4:, :])
                    return wvt

                wvb_t = {}
                wv8_t = {}
                for n in range(2):
                    for hb in range(2):
                        wvb_t[n, hb] = load_wv(wv_pool, wvb_d, n, hb, BF16,
                                               f"wvb{n}{hb}")
                        wv8_t[n, hb] = load_wv(wv8_pool, wv8_d, n, hb, F8,
                                               f"wv8{n}{hb}")

                # Q weight loads: emitted here so they stream during K/V
                wqb = [load_w(wbf_pool, wqb_d, m, BF16, f"wqb{m}")
                       for m in range(DC)]
                wq8 = [load_w(w8_pool, wq8_d, m, F8, f"wq8{m}")
                       for m in range(DC)]

                for n in range(4):
                    if n + 2 < 4:
                        for hb in range(2):
                            wvb_t[n + 2, hb] = load_wv(
                                wv_pool, wvb_d, n + 2, hb, BF16,
                                f"wvb{n + 2}{hb}")
                            wv8_t[n + 2, hb] = load_wv(
                                wv8_pool, wv8_d, n + 2, hb, F8,
                                f"wv8{n + 2}{hb}")
                    # c_lo rows: bf16, dual-staged
                    for s in range(4):
                        ps = ps_all.tile([P, 512], F32, tag="ps")
                        for dc in range(DC):
                            w = wvb_t[n, dc // 8]
                            nc.tensor.matmul(
                                ps[:], lhsT=xtb[:, dc, ts(s, P)],
                                rhs=w[:, dc % 8, :],
                                start=(dc == 0), stop=(dc == DC - 1),
                            )
                        svb = sv_pool.tile([P, 512], BF16, tag="sv")
                        nc.vector.tensor_copy(svb[:], ps[:])
                        sv8 = sv8_pool.tile([P, 512], F8, tag="sv8")
                        nc.scalar.copy(sv8[:], ps[:])
                        nc.scalar.dma_start(
                            vvb_own[s // 2][s % 2, :, ts(n, 512)], svb[:])
                        nc.scalar.dma_start(
                            vv8_own[s // 2][s % 2, :, ts(n, 512)], sv8[:])
                    # c_hi rows: fp8 DoubleRow
                    for s in range(4):
                        ps = ps_all.tile([P, 512], F32, tag="ps")
                        for t in range(8):
                            w = wv8_t[n, t // 4]
                            nc.tensor.matmul(
                                ps[:], lhsT=xt8[:, ds(2 * t, 2), ts(s, P)],
                                rhs=w[:, ds(2 * (t % 4), 2), :],
                                start=(t == 0), stop=(t == 7), perf_mode=DR,
                            )
                        sv8 = sv8_pool.tile([P, 512], F8, tag="sv8")
                        nc.scalar.activation(sv8[:], ps[:], Copy,
                                             scale=1.0 / WS)
                        nc.scalar.dma_start(
                            vv8_own[2 + s // 2][s % 2, :, ts(n, 512)], sv8[:])

                for g in range(2):
                    nc.gpsimd.collective_compute(
                        "AllGather", mybir.AluOpType.bypass,
                        replica_groups=PAIRS,
                        ins=[vvb_own[g].opt()], outs=[vgb[g].opt()])
                for g in range(4):
                    nc.gpsimd.collective_compute(
                        "AllGather", mybir.AluOpType.bypass,
                        replica_groups=PAIRS,
                        ins=[vv8_own[g].opt()], outs=[vg8[g].opt()])

                # --- Q: c_lo bf16 -> qTb; c_hi fp8 DR -> qT8
                for m in range(DC):
                    ps = ps_all.tile([P, 512], F32, tag="ps")
                    for dc in range(DC):
                        nc.tensor.matmul(
                            ps[:], lhsT=wqb[m][:, dc, :], rhs=xtb[:, dc, :],
                            start=(dc == 0), stop=(dc == DC - 1),
                        )
                    stb = st_pool.tile([P, 512], BF16, tag="st")
                    nc.scalar.copy(stb[:], ps[:])
                    nc.scalar.dma_start(qTb[:, m, :], stb[:])
                for m in range(DC):
                    ps = ps_all.tile([P, 512], F32, tag="ps")
                    for t in range(8):
                        nc.tensor.matmul(
                            ps[:], lhsT=wq8[m][:, ds(2 * t, 2), :],
                            rhs=xt8[:, ds(2 * t, 2), :],
                            start=(t == 0), stop=(t == 7), perf_mode=DR,
                        )
                    st8 = st8_pool.tile([P, 512], F8, tag="st8")
                    nc.scalar.activation(st8[:], ps[:], Copy, scale=1.0 / WS)
                    nc.scalar.dma_start(qT8[:, m, :], st8[:])

            # ---------------- phase 2: attention ----------------
            with (
                tc.tile_pool(name="pt", bufs=1) as pt_pool,
                tc.tile_pool(name="mk", bufs=1) as mk_pool,
                tc.tile_pool(name="vb", bufs=1) as vb_pool,
                tc.tile_pool(name="ktb", bufs=4) as ktb_pool,
                tc.tile_pool(name="kt8", bufs=8) as kt8_pool,
                tc.tile_pool(name="qtv", bufs=1) as qtv_pool,
                tc.tile_pool(name="one", bufs=1) as one_pool,
                tc.tile_pool(name="sc", bufs=4) as sc_pool,
                tc.tile_pool(name="ob", bufs=4) as ob_pool,
            ):
                mkb = mk_pool.tile([P, 8, 512], BF16, tag="mkb")
                nc.sync.dma_start(mkb[:, :4, :], mkb_d.ap()[:, :4, :])
                nc.sync.dma_start(mkb[:, 4:, :], mkb_d.ap()[:, 4:, :])
                mk8 = mk_pool.tile([P, 16, 512], F8, tag="mk8")
                nc.sync.dma_start(mk8[:, :8, :], mk8_d.ap()[:, :8, :])
                nc.sync.dma_start(mk8[:, 8:, :], mk8_d.ap()[:, 8:, :])
                onesb = one_pool.tile([P, 1], BF16, tag="onesb")
                nc.vector.memset(onesb[:], 1.0)
                ones8 = one_pool.tile([P, 2, 1], F8, tag="ones8")
                nc.vector.memset(ones8[:], 1.0)
                ebias = one_pool.tile([P, 1], F32, tag="ebias")
                nc.vector.memset(ebias[:], ESHIFT)

                qtb = qtv_pool.tile([P, DC, 512], BF16, tag="qtb")
                for j in range(4):
                    nc.sync.dma_start(qtb[:, ts(j, 4), :], qTb[:, ts(j, 4), :])
                qt8 = qtv_pool.tile([P, DC, 512], F8, tag="qt8")
                for j in range(4):
                    nc.sync.dma_start(qt8[:, ts(j, 4), :], qT8[:, ts(j, 4), :])

                ptb = pt_pool.tile([P, 8, 512], BF16, tag="ptb")
                pt8 = pt_pool.tile([P, 16, 512], F8, tag="pt8")

                # V big tiles: emitted before the score loop so the 8MB
                # streams in under the score matmuls (the gathers finish
                # well before phase 2)
                vbb = vb_pool.tile([P, 8, D], BF16, tag="vbb")
                for jj, pos in enumerate(SLOT0_POS):
                    nc.sync.dma_start(
                        vbb[:, jj, :],
                        vgb[(pos % 8) // 2][pos // 8, pos % 2])
                v8b = vb_pool.tile([P, 16, D], F8, tag="v8b")
                for pos in range(SB):
                    nc.sync.dma_start(
                        v8b[:, pos, :],
                        vg8[(pos % 8) // 2][pos // 8, pos % 2])

                def kt8_src(pos):
                    lo = pos in (0, 1, 2, 3, 8, 9, 10, 11)
                    return kg8[0 if lo else 1][pos // 8, pos % 4]

                for pos in range(SB):
                    kt8_t = kt8_pool.tile([P, DC, P], F8, tag="kt8",
                                          name=f"kt8_{pos}")
                    for j in range(4):
                        nc.sync.dma_start(kt8_t[:, ts(j, 4), :],
                                          kt8_src(pos)[:, ts(j, 4), :])
                    ps = ps_all.tile([P, 512], F32, tag="ps")
                    for t in range(8):
                        nc.tensor.matmul(
                            ps[:], lhsT=kt8_t[:, ds(2 * t, 2), :],
                            rhs=qt8[:, ds(2 * t, 2), :],
                            start=(t == 0), stop=(t == 7), perf_mode=DR,
                        )
                    nc.scalar.activation(pt8[:, pos, :], ps[:], Exp,
                                         scale=INV_SQRT_D, bias=ebias[:])
                    if pos in S0IDX:
                        ktb_t = ktb_pool.tile([P, DC, P], BF16, tag="ktb",
                                              name=f"ktb_{pos}")
                        for j in range(4):
                            nc.sync.dma_start(
                                ktb_t[:, ts(j, 4), :],
                                kgb[pos // 8, pos % 4][:, ts(j, 4), :])
                        ps2 = ps_all.tile([P, 512], F32, tag="ps")
                        for dc in range(DC):
                            nc.tensor.matmul(
                                ps2[:], lhsT=ktb_t[:, dc, :],
                                rhs=qtb[:, dc, :],
                                start=(dc == 0), stop=(dc == DC - 1),
                            )
                        nc.scalar.activation(ptb[:, S0IDX[pos], :], ps2[:],
                                             Exp, scale=INV_SQRT_D)

                for j in range(8):
                    nc.vector.tensor_mul(ptb[:, j, :], ptb[:, j, :],
                                         mkb[:, j, :])
                for pos in range(SB):
                    nc.vector.tensor_mul(pt8[:, pos, :], pt8[:, pos, :],
                                         mk8[:, pos, :])

                # --- slot 0 (bf16): rowsum, AV, normalize, out rows 0..511
                for qs in range(4):
                    kpos = [0, 1, 2, 3] + list(range(8, 9 + qs))
                    idx = [S0IDX[p] for p in kpos]
                    plt = ps_all.tile([P, 512], F32, tag="ps", name="pl")
                    pl = plt[:, :1]
                    for i, j in enumerate(idx):
                        nc.tensor.matmul(
                            pl[:], lhsT=ptb[:, j, ts(qs, P)], rhs=onesb[:],
                            start=(i == 0), stop=(i == len(idx) - 1),
                        )
                    rl = sc_pool.tile([P, 1], F32, tag="rl")
                    nc.vector.reciprocal(rl[:], pl[:])
                    for n in range(4):
                        pav = ps_all.tile([P, 512], F32, tag="ps", name="pav")
                        for i, j in enumerate(idx):
                            nc.tensor.matmul(
                                pav[:], lhsT=ptb[:, j, ts(qs, P)],
                                rhs=vbb[:, j, ts(n, 512)],
                                start=(i == 0), stop=(i == len(idx) - 1),
                            )
                        ob = ob_pool.tile([P, 512], F32, tag="ob")
                        if n % 2 == 0:
                            nc.vector.tensor_scalar_mul(ob[:], pav[:], rl[:])
                        else:
                            nc.scalar.activation(ob[:], pav[:], Copy,
                                                 scale=rl[:])
                        nc.sync.dma_start(
                            out_d.ap()[ds(qs * P, P), ts(n, 512)], ob[:])

                # --- slot 1 (fp8 DR): rows 512..1023
                for qs in range(4):
                    run1 = 6 if qs < 2 else 8   # [0..4+qs] rounded to even
                    pstarts = list(range(0, run1, 2)) + [8, 10, 12, 14]
                    plt = ps_all.tile([P, 512], F32, tag="ps", name="pl8")
                    pl = plt[:, :1]
                    for i, p0 in enumerate(pstarts):
                        nc.tensor.matmul(
                            pl[:], lhsT=pt8[:, ds(p0, 2), ts(qs, P)],
                            rhs=ones8[:],
                            start=(i == 0), stop=(i == len(pstarts) - 1),
                            perf_mode=DR,
                        )
                    rl = sc_pool.tile([P, 1], F32, tag="rl")
                    nc.vector.reciprocal(rl[:], pl[:])
                    for n in range(4):
                        pav = ps_all.tile([P, 512], F32, tag="ps", name="pav8")
                        for i, p0 in enumerate(pstarts):
                            nc.tensor.matmul(
                                pav[:], lhsT=pt8[:, ds(p0, 2), ts(qs, P)],
                                rhs=v8b[:, ds(p0, 2), ts(n, 512)],
                                start=(i == 0), stop=(i == len(pstarts) - 1),
                                perf_mode=DR,
                            )
                        ob = ob_pool.tile([P, 512], F32, tag="ob")
                        if n % 2 == 0:
                            nc.vector.tensor_scalar_mul(ob[:], pav[:], rl[:])
                        else:
                            nc.scalar.activation(ob[:], pav[:], Copy,
                                                 scale=rl[:])
                        nc.sync.dma_start(
                            out_d.ap()[ds(512 + qs * P, P), ts(n, 512)], ob[:])

    nc.compile()
    _CACHED_NC = nc
    return nc


def _host_prep(x, Wq, Wk, Wv):
    """Build per-core input maps (host-side layout prep)."""
    f8 = ml_dtypes.float8_e4m3
    bf = ml_dtypes.bfloat16

    def wqk_layout(W, dt, scale=1.0):
        return np.ascontiguousarray(
            (W * scale).reshape(DC, P, DC, P).transpose(2, 1, 0, 3)).astype(dt)

    def wv_layout(W, dt, scale=1.0):
        return np.ascontiguousarray(
            (W * scale).reshape(2, 8, P, 4, 512).transpose(3, 0, 2, 1, 4)
        ).astype(dt)

    wqb_h = wqk_layout(Wq, bf)
    wq8_h = wqk_layout(Wq, f8, WS)
    wkb_h = wqk_layout(Wk, bf)
    wk8_h = wqk_layout(Wk, f8, WS)
    wvb_h = wv_layout(Wv, bf)
    wv8_h = wv_layout(Wv, f8, WS)

    k_in_block = np.arange(P, dtype=np.int64)[:, None]           # [P, 1]
    q_in_chunk = np.arange(512, dtype=np.int64)[None, :]         # [1, 512]

    def build_masks(h):
        c_lo, c_hi = h, 3 - h
        mb = np.zeros((P, 8, 512), dtype=bf)
        for j, pos in enumerate(SLOT0_POS):
            tkb = POS2TRUE[pos // 4] * 4 + pos % 4
            mb[:, j, :] = (tkb * P + k_in_block) <= (c_lo * 512 + q_in_chunk)
        m8 = np.zeros((P, 16, 512), dtype=f8)
        for pos in range(SB):
            tkb = POS2TRUE[pos // 4] * 4 + pos % 4
            m8[:, pos, :] = (tkb * P + k_in_block) <= (c_hi * 512 + q_in_chunk)
        return mb, m8

    mask_h = [build_masks(0), build_masks(1)]

    in_maps = []
    for core in range(NCORES):
        b, h = divmod(core, 2)
        c_lo, c_hi = h, 3 - h
        xt = x[b].T                                               # [D, S] view
        in_maps.append({
            "xtb": np.ascontiguousarray(
                xt[:, c_lo * 512:(c_lo + 1) * 512]).astype(bf),
            "xt8": np.ascontiguousarray(
                xt[:, c_hi * 512:(c_hi + 1) * 512]).astype(f8),
            "wqb": wqb_h, "wq8": wq8_h, "wkb": wkb_h, "wk8": wk8_h,
            "wvb": wvb_h, "wv8": wv8_h,
            "maskb": mask_h[h][0], "mask8": mask_h[h][1],
        })
    return in_maps


def run(x, Wq, Wk, Wv, trace=False):
    x = np.asarray(x, dtype=np.float32)
    Wq = np.asarray(Wq, dtype=np.float32)
    Wk = np.asarray(Wk, dtype=np.float32)
    Wv = np.asarray(Wv, dtype=np.float32)
    nc = build_nc()
    in_maps = _host_prep(x, Wq, Wk, Wv)
    res = run_bass_kernel_spmd(nc, in_maps, core_ids=list(range(NCORES)),
                               trace=trace)
    out = np.empty((B, S, D), dtype=np.float32)
    for core in range(NCORES):
        b, h = divmod(core, 2)
        c_lo, c_hi = h, 3 - h
        o = res.results[core]["out"]
        out[b, c_lo * 512:(c_lo + 1) * 512] = o[:512]
        out[b, c_hi * 512:(c_hi + 1) * 512] = o[512:]
    return out, res


def kernel(x, Wq, Wk, Wv):
    out, _ = run(x, Wq, Wk, Wv)
    return out


if __name__ == "__main__":
    build_nc()
    print("build + compile OK")


# revision 17
# speedup vs baseline: 1.1840x; 1.1840x over previous
"""Causal attention kernel for Trainium2, 8 NeuronCores — depth-split fp8.

Problem: x[4,2048,2048] @ Wq/Wk/Wv[2048,2048] -> causal softmax attention.

Sharding (as baseline): 2 cores per batch; each core owns 1024 query rows as
global 512-row chunks {0,3} (even cores) / {1,2} (odd cores). Pairwise
AllGather assembles full K^T / V per batch. Gathered key-block positions are
[chunk0, chunk3, chunk1, chunk2] on every core; query slot 0 (c_lo) attends
positions {0-3, 8-11}, slot 1 (c_hi) all 16; causality via mask tensors.

Depth-split fp8: rows in chunks 2,3 (slot 1, >=1025 visible keys) have
diffuse softmax, so fp8-e4m3 noise (~4%/tensor) attenuates to <1% there:
  - slot-1 Q/K/V projections in fp8 DoubleRow (2 fp8/PE cell, 256-deep
    contraction per pass), weights host-scaled x64, staged back at 1/64.
  - slot-1 scores fp8 DoubleRow; exp shifted by -2 so unnormalized fp8
    probs stay < 240 (TRN e4m3 max); shift cancels in normalization.
  - slot-1 AV in fp8 DoubleRow (adjacent key-block pairs; odd-length
    causal runs rounded up — the padded block's probs are mask-zeroed).
  - slot-0 (chunks 0,1) stays bf16 end-to-end: shallow rows concentrate
    softmax mass and pass v through nearly verbatim (sim: bf16 4.9e-3,
    full-fp8 4.5e-2, this split 1.33e-2 vs the 2e-2 gate).
c_lo K/V are computed in bf16 and dual-staged (bf16 + fp8 copy); c_hi rows
never need bf16 x.

Scheduling: staging stores ride the gpsimd queue (engine-local DIRECT2D);
the 9 collectives are emitted so nothing tensor-critical queues behind
their peer-sync waits — in particular Q staging writes PSUM->SBUF straight
into the phase-2 q tiles (no DRAM round-trip, no gpsimd dependency).
First x/W tiles are split small so the first matmul waits on ~0.75MB.
"""

import math

import numpy as np
import ml_dtypes

import concourse.bass as bass
import concourse.mybir as mybir
import concourse.tile as tile
from concourse import bacc
from concourse.bass import ds, ts
from concourse.bass_utils import run_bass_kernel_spmd

B, S, D = 4, 2048, 2048
P = 128
DC = D // P          # 16 contraction chunks
SB = S // P          # 16 key blocks
QROWS = 1024         # query rows per core
NCORES = 8
INV_SQRT_D = 1.0 / math.sqrt(D)
WS = 64.0            # host scale on fp8 weights (keeps W8 in e4m3 normal range)
ESHIFT = -2.0        # exp shift: unnormalized fp8 probs < 240

# gathered key-block position -> true 512-chunk (pair-rank order, all cores)
POS2TRUE = [0, 3, 1, 2]
SLOT0_POS = [0, 1, 2, 3, 8, 9, 10, 11]   # slot-0's (bf16) key positions
S0IDX = {pos: j for j, pos in enumerate(SLOT0_POS)}
PAIRS = [[0, 1], [2, 3], [4, 5], [6, 7]]

F32 = mybir.dt.float32
BF16 = mybir.dt.bfloat16
F8 = mybir.dt.float8e4
DR = mybir.MatmulPerfMode.DoubleRow
Exp = mybir.ActivationFunctionType.Exp
Copy = mybir.ActivationFunctionType.Copy

_CACHED_NC = None


def build_nc():
    global _CACHED_NC
    if _CACHED_NC is not None:
        return _CACHED_NC
    nc = bacc.Bacc(trn_type="TRN2", target_bir_lowering=False, debug=False,
                   num_devices=NCORES)

    xtb_d = nc.dram_tensor("xtb", [D, 512], BF16, kind="ExternalInput")
    xt8_d = nc.dram_tensor("xt8", [D, 512], F8, kind="ExternalInput")
    wqb_d = nc.dram_tensor("wqb", [DC, P, DC, P], BF16, kind="ExternalInput")
    wq8_d = nc.dram_tensor("wq8", [DC, P, DC, P], F8, kind="ExternalInput")
    wkb_d = nc.dram_tensor("wkb", [DC, P, DC, P], BF16, kind="ExternalInput")
    wk8_d = nc.dram_tensor("wk8", [DC, P, DC, P], F8, kind="ExternalInput")
    wvb_d = nc.dram_tensor("wvb", [4, 2, P, 8, 512], BF16, kind="ExternalInput")
    wv8_d = nc.dram_tensor("wv8", [4, 2, P, 8, 512], F8, kind="ExternalInput")
    mkb_d = nc.dram_tensor("maskb", [P, 8, 512], BF16, kind="ExternalInput")
    mk8_d = nc.dram_tensor("mask8", [P, 16, 512], F8, kind="ExternalInput")
    out_d = nc.dram_tensor("out", [QROWS, D], F32, kind="ExternalOutput")

    with tile.TileContext(nc) as tc:
        with (
            tc.tile_pool(name="dram", bufs=1, space="DRAM") as dpool,
            tc.tile_pool(name="ps", bufs=8, space="PSUM") as ps_all,
            tc.tile_pool(name="qsb", bufs=1) as qsb_pool,
        ):
            # phase-2 q tiles, written directly by Q staging (PSUM->SBUF)
            qtb = qsb_pool.tile([P, DC, 512], BF16, tag="qtb")
            qt8 = qsb_pool.tile([P, DC, 512], F8, tag="qt8")

            kTb_own = dpool.tile([4, P, DC, P], BF16, tag="kTbo")
            kT8_own = [dpool.tile([4, P, DC, P], F8, tag=f"kT8o{s}",
                                  name=f"kT8o{s}") for s in range(2)]
            kgb = dpool.tile([2, 4, P, DC, P], BF16, tag="kgb")
            kg8 = [dpool.tile([2, 4, P, DC, P], F8, tag=f"kg8{s}",
                              name=f"kg8{s}") for s in range(2)]
            vvb_own = [dpool.tile([2, P, D], BF16, tag=f"vvbo{g}",
                                  name=f"vvbo{g}") for g in range(2)]
            vv8_own = [dpool.tile([2, P, D], F8, tag=f"vv8o{g}",
                                  name=f"vv8o{g}") for g in range(4)]
            vgb = [dpool.tile([2, 2, P, D], BF16, tag=f"vgb{g}",
                              name=f"vgb{g}") for g in range(2)]
            vg8 = [dpool.tile([2, 2, P, D], F8, tag=f"vg8{g}",
                              name=f"vg8{g}") for g in range(4)]

            # ---------------- phase 1: projections ----------------
            with (
                tc.tile_pool(name="xt", bufs=1) as xt_pool,
                tc.tile_pool(name="wbf", bufs=20) as wbf_pool,
                tc.tile_pool(name="w8", bufs=16) as w8_pool,
                tc.tile_pool(name="wv", bufs=5) as wv_pool,
                tc.tile_pool(name="wv8", bufs=4) as wv8_pool,
                tc.tile_pool(name="st", bufs=8) as st_pool,
                tc.tile_pool(name="st8", bufs=8) as st8_pool,
                tc.tile_pool(name="sv", bufs=5) as sv_pool,
                tc.tile_pool(name="sv8", bufs=6) as sv8_pool,
            ):
                # bf16 W tiles split in half so the first matmul's deps
                # are small (dep tracking is tile-granular)
                def load_wb(dram, m, name):
                    lo = wbf_pool.tile([P, 8, P], BF16, tag="w",
                                       name=f"{name}l")
                    nc.sync.dma_start(lo[:], dram.ap()[m][:, :8, :])
                    hi = wbf_pool.tile([P, 8, P], BF16, tag="w",
                                       name=f"{name}h")
                    nc.sync.dma_start(hi[:], dram.ap()[m][:, 8:, :])
                    return lo, hi

                def wb_ap(wpair, dc):
                    return wpair[dc // 8][:, dc % 8, :]

                def load_w8(dram, m, name):
                    wt = w8_pool.tile([P, DC, P], F8, tag="w", name=name)
                    nc.sync.dma_start(wt[:, :8, :], dram.ap()[m][:, :8, :])
                    nc.sync.dma_start(wt[:, 8:, :], dram.ap()[m][:, 8:, :])
                    return wt

                wkb = [load_wb(wkb_d, 0, "wkb0")]
                # x^T c_lo split into 4 tiles so the first matmuls' deps
                # are 0.5MB each
                xtbs = []
                for c in range(4):
                    t = xt_pool.tile([P, 4, 512], BF16, tag=f"xtb{c}",
                                     name=f"xtb{c}")
                    for i in range(4):
                        nc.sync.dma_start(
                            t[:, i, :], xtb_d.ap()[ds((4 * c + i) * P, P), :])
                    xtbs.append(t)

                def xtb_ap(dc):
                    return xtbs[dc // 4][:, dc % 4, :]

                wkb += [load_wb(wkb_d, m, f"wkb{m}") for m in range(1, 8)]
                xt8 = xt_pool.tile([P, DC, 512], F8, tag="xt8")
                for dc in range(DC):
                    nc.sync.dma_start(xt8[:, dc, :], xt8_d.ap()[ds(dc * P, P), :])
                wkb += [load_wb(wkb_d, m, f"wkb{m}") for m in range(8, DC)]
                wk8 = [load_w8(wk8_d, m, f"wk8{m}") for m in range(DC)]

                # --- K c_lo rows: bf16, dual-staged (bf16 + fp8)
                for m in range(DC):
                    ps = ps_all.tile([P, 512], F32, tag="ps")
                    for dc in range(DC):
                        nc.tensor.matmul(
                            ps[:], lhsT=wb_ap(wkb[m], dc), rhs=xtb_ap(dc),
                            start=(dc == 0), stop=(dc == DC - 1),
                        )
                    stb = st_pool.tile([P, 512], BF16, tag="st")
                    nc.scalar.copy(stb[:], ps[:])
                    st8 = st8_pool.tile([P, 512], F8, tag="st8")
                    nc.vector.tensor_copy(st8[:], ps[:])
                    for j in range(4):
                        nc.gpsimd.dma_start(kTb_own[j, :, m, :], stb[:, ts(j, P)])
                        nc.gpsimd.dma_start(kT8_own[0][j, :, m, :],
                                            st8[:, ts(j, P)])
                nc.gpsimd.collective_compute(
                    "AllGather", mybir.AluOpType.bypass, replica_groups=PAIRS,
                    ins=[kTb_own.opt()], outs=[kgb.opt()])
                nc.gpsimd.collective_compute(
                    "AllGather", mybir.AluOpType.bypass, replica_groups=PAIRS,
                    ins=[kT8_own[0].opt()], outs=[kg8[0].opt()])

                # --- K c_hi rows: fp8 DoubleRow
                for m in range(DC):
                    ps = ps_all.tile([P, 512], F32, tag="ps")
                    for t in range(8):
                        nc.tensor.matmul(
                            ps[:], lhsT=wk8[m][:, ds(2 * t, 2), :],
                            rhs=xt8[:, ds(2 * t, 2), :],
                            start=(t == 0), stop=(t == 7), perf_mode=DR,
                        )
                    st8 = st8_pool.tile([P, 512], F8, tag="st8")
                    nc.scalar.activation(st8[:], ps[:], Copy, scale=1.0 / WS)
                    for j in range(4):
                        nc.gpsimd.dma_start(kT8_own[1][j, :, m, :],
                                            st8[:, ts(j, P)])
                nc.gpsimd.collective_compute(
                    "AllGather", mybir.AluOpType.bypass, replica_groups=PAIRS,
                    ins=[kT8_own[1].opt()], outs=[kg8[1].opt()])

                # --- V: n-outer so only 2 wv tile pairs are resident
                def load_wv(pool, dram, n, hb, dt, name):
                    wvt = pool.tile([P, 8, 512], dt, tag="wv", name=name)
                    nc.sync.dma_start(wvt[:, :4, :], dram.ap()[n, hb][:, :4, :])
                    nc.sync.dma_start(wvt[:, 4:, :], dram.ap()[n, hb][:, 4:, :])
                    return wvt

                wvb_t = {}
                wv8_t = {}
                for n in range(2):
                    for hb in range(2):
                        wvb_t[n, hb] = load_wv(wv_pool, wvb_d, n, hb, BF16,
                                               f"wvb{n}{hb}")
                        wv8_t[n, hb] = load_wv(wv8_pool, wv8_d, n, hb, F8,
                                               f"wv8{n}{hb}")

                # Q weight loads: emitted here so they stream during K/V
                wqb = [load_wb(wqb_d, m, f"wqb{m}") for m in range(DC)]
                wq8 = [load_w8(wq8_d, m, f"wq8{m}") for m in range(DC)]

                for n in range(4):
                    if n + 2 < 4:
                        for hb in range(2):
                            wvb_t[n + 2, hb] = load_wv(
                                wv_pool, wvb_d, n + 2, hb, BF16,
                                f"wvb{n + 2}{hb}")
                            wv8_t[n + 2, hb] = load_wv(
                                wv8_pool, wv8_d, n + 2, hb, F8,
                                f"wv8{n + 2}{hb}")
                    # c_lo rows: bf16, dual-staged
                    for s in range(4):
                        ps = ps_all.tile([P, 512], F32, tag="ps")
                        for dc in range(DC):
                            w = wvb_t[n, dc // 8]
                            nc.tensor.matmul(
                                ps[:], lhsT=xtb_ap(dc)[:, ts(s, P)],
                                rhs=w[:, dc % 8, :],
                                start=(dc == 0), stop=(dc == DC - 1),
                            )
                        svb = sv_pool.tile([P, 512], BF16, tag="sv")
                        nc.vector.tensor_copy(svb[:], ps[:])
                        sv8 = sv8_pool.tile([P, 512], F8, tag="sv8")
                        nc.scalar.copy(sv8[:], ps[:])
                        nc.gpsimd.dma_start(
                            vvb_own[s // 2][s % 2, :, ts(n, 512)], svb[:])
                        nc.gpsimd.dma_start(
                            vv8_own[s // 2][s % 2, :, ts(n, 512)], sv8[:])
                    # c_hi rows: fp8 DoubleRow
                    for s in range(4):
                        ps = ps_all.tile([P, 512], F32, tag="ps")
                        for t in range(8):
                            w = wv8_t[n, t // 4]
                            nc.tensor.matmul(
                                ps[:], lhsT=xt8[:, ds(2 * t, 2), ts(s, P)],
                                rhs=w[:, ds(2 * (t % 4), 2), :],
                                start=(t == 0), stop=(t == 7), perf_mode=DR,
                            )
                        sv8 = sv8_pool.tile([P, 512], F8, tag="sv8")
                        nc.scalar.activation(sv8[:], ps[:], Copy,
                                             scale=1.0 / WS)
                        nc.gpsimd.dma_start(
                            vv8_own[2 + s // 2][s % 2, :, ts(n, 512)], sv8[:])

                # --- Q: c_lo bf16 / c_hi fp8 DR, staged straight into the
                # phase-2 SBUF q tiles (no DRAM round-trip, no gpsimd dep:
                # the V AllGathers below can't stall Q)
                for m in range(DC):
                    ps = ps_all.tile([P, 512], F32, tag="ps")
                    for dc in range(DC):
                        nc.tensor.matmul(
                            ps[:], lhsT=wb_ap(wqb[m], dc), rhs=xtb_ap(dc),
                            start=(dc == 0), stop=(dc == DC - 1),
                        )
                    nc.scalar.copy(qtb[:, m, :], ps[:])
                for m in range(DC):
                    ps = ps_all.tile([P, 512], F32, tag="ps")
                    for t in range(8):
                        nc.tensor.matmul(
                            ps[:], lhsT=wq8[m][:, ds(2 * t, 2), :],
                            rhs=xt8[:, ds(2 * t, 2), :],
                            start=(t == 0), stop=(t == 7), perf_mode=DR,
                        )
                    nc.scalar.activation(qt8[:, m, :], ps[:], Copy,
                                         scale=1.0 / WS)

                for g in range(2):
                    nc.gpsimd.collective_compute(
                        "AllGather", mybir.AluOpType.bypass,
                        replica_groups=PAIRS,
                        ins=[vvb_own[g].opt()], outs=[vgb[g].opt()])
                for g in range(4):
                    nc.gpsimd.collective_compute(
                        "AllGather", mybir.AluOpType.bypass,
                        replica_groups=PAIRS,
                        ins=[vv8_own[g].opt()], outs=[vg8[g].opt()])

            # ---------------- phase 2: attention ----------------
            with (
                tc.tile_pool(name="pt", bufs=1) as pt_pool,
                tc.tile_pool(name="mk", bufs=1) as mk_pool,
                tc.tile_pool(name="vb", bufs=1) as vb_pool,
                tc.tile_pool(name="ktb", bufs=4) as ktb_pool,
                tc.tile_pool(name="kt8", bufs=8) as kt8_pool,
                tc.tile_pool(name="one", bufs=1) as one_pool,
                tc.tile_pool(name="sc", bufs=4) as sc_pool,
                tc.tile_pool(name="ob", bufs=4) as ob_pool,
            ):
                mkb = mk_pool.tile([P, 8, 512], BF16, tag="mkb")
                nc.sync.dma_start(mkb[:, :4, :], mkb_d.ap()[:, :4, :])
                nc.sync.dma_start(mkb[:, 4:, :], mkb_d.ap()[:, 4:, :])
                mk8 = mk_pool.tile([P, 16, 512], F8, tag="mk8")
                nc.sync.dma_start(mk8[:, :8, :], mk8_d.ap()[:, :8, :])
                nc.sync.dma_start(mk8[:, 8:, :], mk8_d.ap()[:, 8:, :])
                onesb = one_pool.tile([P, 1], BF16, tag="onesb")
                nc.vector.memset(onesb[:], 1.0)
                ones8 = one_pool.tile([P, 2, 1], F8, tag="ones8")
                nc.vector.memset(ones8[:], 1.0)
                ebias = one_pool.tile([P, 1], F32, tag="ebias")
                nc.vector.memset(ebias[:], ESHIFT)

                ptb = pt_pool.tile([P, 8, 512], BF16, tag="ptb")
                pt8 = pt_pool.tile([P, 16, 512], F8, tag="pt8")

                # V big tiles: emitted before the score loop so the 8MB
                # streams in under the score matmuls
                vbb = vb_pool.tile([P, 8, D], BF16, tag="vbb")
                for jj, pos in enumerate(SLOT0_POS):
                    nc.sync.dma_start(
                        vbb[:, jj, :],
                        vgb[(pos % 8) // 2][pos // 8, pos % 2])
                v8b = vb_pool.tile([P, 16, D], F8, tag="v8b")
                for pos in range(SB):
                    nc.sync.dma_start(
                        v8b[:, pos, :],
                        vg8[(pos % 8) // 2][pos // 8, pos % 2])

                def kt8_src(pos):
                    lo = pos in (0, 1, 2, 3, 8, 9, 10, 11)
                    return kg8[0 if lo else 1][pos // 8, pos % 4]

                for pos in range(SB):
                    kt8_t = kt8_pool.tile([P, DC, P], F8, tag="kt8",
                                          name=f"kt8_{pos}")
                    for j in range(4):
                        nc.sync.dma_start(kt8_t[:, ts(j, 4), :],
                                          kt8_src(pos)[:, ts(j, 4), :])
                    ps = ps_all.tile([P, 512], F32, tag="ps")
                    for t in range(8):
                        nc.tensor.matmul(
                            ps[:], lhsT=kt8_t[:, ds(2 * t, 2), :],
                            rhs=qt8[:, ds(2 * t, 2), :],
                            start=(t == 0), stop=(t == 7), perf_mode=DR,
                        )
                    nc.scalar.activation(pt8[:, pos, :], ps[:], Exp,
                                         scale=INV_SQRT_D, bias=ebias[:])
                    if pos in S0IDX:
                        ktb_t = ktb_pool.tile([P, DC, P], BF16, tag="ktb",
                                              name=f"ktb_{pos}")
                        for j in range(4):
                            nc.sync.dma_start(
                                ktb_t[:, ts(j, 4), :],
                                kgb[pos // 8, pos % 4][:, ts(j, 4), :])
                        ps2 = ps_all.tile([P, 512], F32, tag="ps")
                        for dc in range(DC):
                            nc.tensor.matmul(
                                ps2[:], lhsT=ktb_t[:, dc, :],
                                rhs=qtb[:, dc, :],
                                start=(dc == 0), stop=(dc == DC - 1),
                            )
                        nc.scalar.activation(ptb[:, S0IDX[pos], :], ps2[:],
                                             Exp, scale=INV_SQRT_D)

                for j in range(8):
                    nc.vector.tensor_mul(ptb[:, j, :], ptb[:, j, :],
                                         mkb[:, j, :])
                for pos in range(SB):
                    nc.vector.tensor_mul(pt8[:, pos, :], pt8[:, pos, :],
                                         mk8[:, pos, :])

                # --- slot 0 (bf16): rowsum, AV, normalize, out rows 0..511
                for qs in range(4):
                    kpos = [0, 1, 2, 3] + list(range(8, 9 + qs))
                    idx = [S0IDX[p] for p in kpos]
                    plt = ps_all.tile([P, 512], F32, tag="ps", name="pl")
                    pl = plt[:, :1]
                    for i, j in enumerate(idx):
                        nc.tensor.matmul(
                            pl[:], lhsT=ptb[:, j, ts(qs, P)], rhs=onesb[:],
                            start=(i == 0), stop=(i == len(idx) - 1),
                        )
                    rl = sc_pool.tile([P, 1], F32, tag="rl")
                    nc.vector.reciprocal(rl[:], pl[:])
                    for n in range(4):
                        pav = ps_all.tile([P, 512], F32, tag="ps", name="pav")
                        for i, j in enumerate(idx):
                            nc.tensor.matmul(
                                pav[:], lhsT=ptb[:, j, ts(qs, P)],
                                rhs=vbb[:, j, ts(n, 512)],
                                start=(i == 0), stop=(i == len(idx) - 1),
                            )
                        ob = ob_pool.tile([P, 512], F32, tag="ob")
                        if n % 2 == 0:
                            nc.vector.tensor_scalar_mul(ob[:], pav[:], rl[:])
                        else:
                            nc.scalar.activation(ob[:], pav[:], Copy,
                                                 scale=rl[:])
                        nc.sync.dma_start(
                            out_d.ap()[ds(qs * P, P), ts(n, 512)], ob[:])

                # --- slot 1 (fp8 DR): rows 512..1023
                for qs in range(4):
                    run1 = 6 if qs < 2 else 8   # [0..4+qs] rounded to even
                    pstarts = list(range(0, run1, 2)) + [8, 10, 12, 14]
                    plt = ps_all.tile([P, 512], F32, tag="ps", name="pl8")
                    pl = plt[:, :1]
                    for i, p0 in enumerate(pstarts):
                        nc.tensor.matmul(
                            pl[:], lhsT=pt8[:, ds(p0, 2), ts(qs, P)],
                            rhs=ones8[:],
                            start=(i == 0), stop=(i == len(pstarts) - 1),
                            perf_mode=DR,
                        )
                    rl = sc_pool.tile([P, 1], F32, tag="rl")
                    nc.vector.reciprocal(rl[:], pl[:])
                    for n in range(4):
                        pav = ps_all.tile([P, 512], F32, tag="ps", name="pav8")
                        for i, p0 in enumerate(pstarts):
                            nc.tensor.matmul(
                                pav[:], lhsT=pt8[:, ds(p0, 2), ts(qs, P)],
                                rhs=v8b[:, ds(p0, 2), ts(n, 512)],
                                start=(i == 0), stop=(i == len(pstarts) - 1),
                                perf_mode=DR,
                            )
                        ob = ob_pool.tile([P, 512], F32, tag="ob")
                        if n % 2 == 0:
                            nc.vector.tensor_scalar_mul(ob[:], pav[:], rl[:])
                        else:
                            nc.scalar.activation(ob[:], pav[:], Copy,
                                                 scale=rl[:])
                        nc.sync.dma_start(
                            out_d.ap()[ds(512 + qs * P, P), ts(n, 512)], ob[:])

    nc.compile()
    _CACHED_NC = nc
    return nc


def _host_prep(x, Wq, Wk, Wv):
    """Build per-core input maps (host-side layout prep)."""
    f8 = ml_dtypes.float8_e4m3
    bf = ml_dtypes.bfloat16

    def wqk_layout(W, dt, scale=1.0):
        return np.ascontiguousarray(
            (W * scale).reshape(DC, P, DC, P).transpose(2, 1, 0, 3)).astype(dt)

    def wv_layout(W, dt, scale=1.0):
        return np.ascontiguousarray(
            (W * scale).reshape(2, 8, P, 4, 512).transpose(3, 0, 2, 1, 4)
        ).astype(dt)

    wqb_h = wqk_layout(Wq, bf)
    wq8_h = wqk_layout(Wq, f8, WS)
    wkb_h = wqk_layout(Wk, bf)
    wk8_h = wqk_layout(Wk, f8, WS)
    wvb_h = wv_layout(Wv, bf)
    wv8_h = wv_layout(Wv, f8, WS)

    k_in_block = np.arange(P, dtype=np.int64)[:, None]           # [P, 1]
    q_in_chunk = np.arange(512, dtype=np.int64)[None, :]         # [1, 512]

    def build_masks(h):
        c_lo, c_hi = h, 3 - h
        mb = np.zeros((P, 8, 512), dtype=bf)
        for j, pos in enumerate(SLOT0_POS):
            tkb = POS2TRUE[pos // 4] * 4 + pos % 4
            mb[:, j, :] = (tkb * P + k_in_block) <= (c_lo * 512 + q_in_chunk)
        m8 = np.zeros((P, 16, 512), dtype=f8)
        for pos in range(SB):
            tkb = POS2TRUE[pos // 4] * 4 + pos % 4
            m8[:, pos, :] = (tkb * P + k_in_block) <= (c_hi * 512 + q_in_chunk)
        return mb, m8

    mask_h = [build_masks(0), build_masks(1)]

    in_maps = []
    for core in range(NCORES):
        b, h = divmod(core, 2)
        c_lo, c_hi = h, 3 - h
        xt = x[b].T                                               # [D, S] view
        in_maps.append({
            "xtb": np.ascontiguousarray(
                xt[:, c_lo * 512:(c_lo + 1) * 512]).astype(bf),
            "xt8": np.ascontiguousarray(
                xt[:, c_hi * 512:(c_hi + 1) * 512]).astype(f8),
            "wqb": wqb_h, "wq8": wq8_h, "wkb": wkb_h, "wk8": wk8_h,
            "wvb": wvb_h, "wv8": wv8_h,
            "maskb": mask_h[h][0], "mask8": mask_h[h][1],
        })
    return in_maps


def run(x, Wq, Wk, Wv, trace=False):
    x = np.asarray(x, dtype=np.float32)
    Wq = np.asarray(Wq, dtype=np.float32)
    Wk = np.asarray(Wk, dtype=np.float32)
    Wv = np.asarray(Wv, dtype=np.float32)
    nc = build_nc()
    in_maps = _host_prep(x, Wq, Wk, Wv)
    res = run_bass_kernel_spmd(nc, in_maps, core_ids=list(range(NCORES)),
                               trace=trace)
    out = np.empty((B, S, D), dtype=np.float32)
    for core in range(NCORES):
        b, h = divmod(core, 2)
        c_lo, c_hi = h, 3 - h
        o = res.results[core]["out"]
        out[b, c_lo * 512:(c_lo + 1) * 512] = o[:512]
        out[b, c_hi * 512:(c_hi + 1) * 512] = o[512:]
    return out, res


def kernel(x, Wq, Wk, Wv):
    out, _ = run(x, Wq, Wk, Wv)
    return out


if __name__ == "__main__":
    build_nc()
    print("build + compile OK")


# revision 27
# speedup vs baseline: 1.2152x; 1.0263x over previous
"""Causal attention kernel for Trainium2, 8 NeuronCores — depth-split fp8.

Problem: x[4,2048,2048] @ Wq/Wk/Wv[2048,2048] -> causal softmax attention.

Sharding (as baseline): 2 cores per batch; each core owns 1024 query rows as
global 512-row chunks {0,3} (even cores) / {1,2} (odd cores). Pairwise
AllGather assembles full K^T / V per batch. Gathered key-block positions are
[chunk0, chunk3, chunk1, chunk2] on every core; query slot 0 (c_lo) attends
positions {0-3, 8-11}, slot 1 (c_hi) all 16; causality via mask tensors.

Depth-split fp8: rows in chunks 2,3 (slot 1, >=1025 visible keys) have
diffuse softmax, so fp8-e4m3 noise (~4%/tensor) attenuates to <1% there:
  - slot-1 Q/K/V projections in fp8 DoubleRow (2 fp8/PE cell, 256-deep
    contraction per pass), weights host-scaled x64, staged back at 1/64.
  - slot-1 scores fp8 DoubleRow; exp shifted by -2 so unnormalized fp8
    probs stay < 240 (TRN e4m3 max); shift cancels in normalization.
  - slot-1 AV in fp8 DoubleRow (adjacent key-block pairs; odd-length
    causal runs rounded up — the padded block's probs are mask-zeroed).
  - slot-0 (chunks 0,1) stays bf16 end-to-end: shallow rows concentrate
    softmax mass and pass v through nearly verbatim (sim: bf16 4.9e-3,
    full-fp8 4.5e-2, this split 1.33e-2 vs the 2e-2 gate).
c_lo K/V are computed in bf16 and dual-staged (bf16 + fp8 copy); c_hi rows
never need bf16 x.

Scheduling: staging stores ride the gpsimd queue (engine-local DIRECT2D);
the 9 collectives are emitted so nothing tensor-critical queues behind
their peer-sync waits — in particular Q staging writes PSUM->SBUF straight
into the phase-2 q tiles (no DRAM round-trip, no gpsimd dependency).
First x/W tiles are split small so the first matmul waits on ~0.75MB.
"""

import math

import numpy as np
import ml_dtypes

import concourse.bass as bass
import concourse.mybir as mybir
import concourse.tile as tile
from concourse import bacc
from concourse.bass import ds, ts
from concourse.bass_utils import run_bass_kernel_spmd

B, S, D = 4, 2048, 2048
P = 128
DC = D // P          # 16 contraction chunks
SB = S // P          # 16 key blocks
QROWS = 1024         # query rows per core
NCORES = 8
INV_SQRT_D = 1.0 / math.sqrt(D)
WS = 64.0            # host scale on fp8 weights (keeps W8 in e4m3 normal range)
ESHIFT = -2.0        # exp shift: unnormalized fp8 probs < 240

# gathered key-block position -> true 512-chunk (pair-rank order, all cores)
POS2TRUE = [0, 3, 1, 2]
SLOT0_POS = [0, 1, 2, 3, 8, 9, 10, 11]   # slot-0's (bf16) key positions
S0IDX = {pos: j for j, pos in enumerate(SLOT0_POS)}
PAIRS = [[0, 1], [2, 3], [4, 5], [6, 7]]

F32 = mybir.dt.float32
BF16 = mybir.dt.bfloat16
F8 = mybir.dt.float8e4
DR = mybir.MatmulPerfMode.DoubleRow
Exp = mybir.ActivationFunctionType.Exp
Copy = mybir.ActivationFunctionType.Copy

_CACHED_NC = None


def build_nc():
    global _CACHED_NC
    if _CACHED_NC is not None:
        return _CACHED_NC
    nc = bacc.Bacc(trn_type="TRN2", target_bir_lowering=False, debug=False,
                   num_devices=NCORES)

    # x^T shipped pre-tiled as [P, DC, 512] so SBUF loads are single DMAs
    xtb_d = nc.dram_tensor("xtb", [P, DC, 512], BF16, kind="ExternalInput")
    xt8_d = nc.dram_tensor("xt8", [P, DC, 512], F8, kind="ExternalInput")
    wqb_d = nc.dram_tensor("wqb", [DC, P, DC, P], BF16, kind="ExternalInput")
    wq8_d = nc.dram_tensor("wq8", [DC, P, DC, P], F8, kind="ExternalInput")
    wkb_d = nc.dram_tensor("wkb", [DC, P, DC, P], BF16, kind="ExternalInput")
    wk8_d = nc.dram_tensor("wk8", [DC, P, DC, P], F8, kind="ExternalInput")
    wvb_d = nc.dram_tensor("wvb", [4, 2, P, 8, 512], BF16, kind="ExternalInput")
    wv8_d = nc.dram_tensor("wv8", [4, 2, P, 8, 512], F8, kind="ExternalInput")
    mkb_d = nc.dram_tensor("maskb", [P, 8, 512], BF16, kind="ExternalInput")
    mk8_d = nc.dram_tensor("mask8", [P, 16, 512], F8, kind="ExternalInput")
    out_d = nc.dram_tensor("out", [QROWS, D], F32, kind="ExternalOutput")

    with tile.TileContext(nc) as tc:
        with (
            tc.tile_pool(name="dram", bufs=1, space="DRAM") as dpool,
            tc.tile_pool(name="ps", bufs=8, space="PSUM") as ps_all,
            tc.tile_pool(name="qsb", bufs=1) as qsb_pool,
        ):
            # phase-2 q tiles, written directly by Q staging (PSUM->SBUF)
            qtb = qsb_pool.tile([P, DC, 512], BF16, tag="qtb")
            qt8 = qsb_pool.tile([P, DC, 512], F8, tag="qt8")

            # [P, DC, 4, P] layout: a whole [P,512] staging tile stores with
            # ONE dma, and a whole [P, DC, P] kt tile loads with one dma
            kTb_own = dpool.tile([P, DC, 4, P], BF16, tag="kTbo")
            kT8_own = [dpool.tile([P, DC, 4, P], F8, tag=f"kT8o{s}",
                                  name=f"kT8o{s}") for s in range(2)]
            kgb = dpool.tile([2, P, DC, 4, P], BF16, tag="kgb")
            kg8 = [dpool.tile([2, P, DC, 4, P], F8, tag=f"kg8{s}",
                              name=f"kg8{s}") for s in range(2)]
            vvb_own = [dpool.tile([2, P, D], BF16, tag=f"vvbo{g}",
                                  name=f"vvbo{g}") for g in range(2)]
            vv8_own = [dpool.tile([2, P, D], F8, tag=f"vv8o{g}",
                                  name=f"vv8o{g}") for g in range(4)]
            vgb = [dpool.tile([2, 2, P, D], BF16, tag=f"vgb{g}",
                              name=f"vgb{g}") for g in range(2)]
            vg8 = [dpool.tile([2, 2, P, D], F8, tag=f"vg8{g}",
                              name=f"vg8{g}") for g in range(4)]

            # ---------------- phase 1: projections ----------------
            with (
                tc.tile_pool(name="xt", bufs=1) as xt_pool,
                tc.tile_pool(name="wbf", bufs=20) as wbf_pool,
                tc.tile_pool(name="w8", bufs=16) as w8_pool,
                tc.tile_pool(name="wv", bufs=5) as wv_pool,
                tc.tile_pool(name="wv8", bufs=4) as wv8_pool,
                tc.tile_pool(name="st", bufs=8) as st_pool,
                tc.tile_pool(name="st8", bufs=8) as st8_pool,
                tc.tile_pool(name="sv", bufs=5) as sv_pool,
                tc.tile_pool(name="sv8", bufs=6) as sv8_pool,
            ):
                # bf16 W tiles split in half so the first matmul's deps
                # are small (dep tracking is tile-granular)
                def load_wb(dram, m, name):
                    lo = wbf_pool.tile([P, 8, P], BF16, tag="w",
                                       name=f"{name}l")
                    nc.sync.dma_start(lo[:], dram.ap()[m][:, :8, :])
                    hi = wbf_pool.tile([P, 8, P], BF16, tag="w",
                                       name=f"{name}h")
                    nc.sync.dma_start(hi[:], dram.ap()[m][:, 8:, :])
                    return lo, hi

                def wb_ap(wpair, dc):
                    return wpair[dc // 8][:, dc % 8, :]

                def load_w8(dram, m, name):
                    wt = w8_pool.tile([P, DC, P], F8, tag="w", name=name)
                    nc.sync.dma_start(wt[:], dram.ap()[m])
                    return wt

                wkb = [load_wb(wkb_d, 0, "wkb0")]
                # x^T c_lo split into 4 tiles so the first matmuls' deps
                # are 0.5MB each
                xtbs = []
                for c in range(4):
                    t = xt_pool.tile([P, 4, 512], BF16, tag=f"xtb{c}",
                                     name=f"xtb{c}")
                    nc.sync.dma_start(t[:], xtb_d.ap()[:, ds(4 * c, 4), :])
                    xtbs.append(t)

                def xtb_ap(dc):
                    return xtbs[dc // 4][:, dc % 4, :]

                wkb += [load_wb(wkb_d, m, f"wkb{m}") for m in range(1, 8)]
                xt8 = xt_pool.tile([P, DC, 512], F8, tag="xt8")
                nc.sync.dma_start(xt8[:, :8, :], xt8_d.ap()[:, :8, :])
                nc.sync.dma_start(xt8[:, 8:, :], xt8_d.ap()[:, 8:, :])
                wkb += [load_wb(wkb_d, m, f"wkb{m}") for m in range(8, DC)]
                wk8 = [load_w8(wk8_d, m, f"wk8{m}") for m in range(DC)]

                # --- K c_lo rows: bf16, dual-staged (bf16 + fp8)
                for m in range(DC):
                    ps = ps_all.tile([P, 512], F32, tag="ps")
                    for dc in range(DC):
                        nc.tensor.matmul(
                            ps[:], lhsT=wb_ap(wkb[m], dc), rhs=xtb_ap(dc),
                            start=(dc == 0), stop=(dc == DC - 1),
                        )
                    stb = st_pool.tile([P, 512], BF16, tag="st")
                    nc.scalar.copy(stb[:], ps[:])
                    st8 = st8_pool.tile([P, 512], F8, tag="st8")
                    nc.vector.tensor_copy(st8[:], ps[:])
                    nc.gpsimd.dma_start(kTb_own[:, m, :, :], stb[:])
                    nc.gpsimd.dma_start(kT8_own[0][:, m, :, :], st8[:])
                nc.gpsimd.collective_compute(
                    "AllGather", mybir.AluOpType.bypass, replica_groups=PAIRS,
                    ins=[kTb_own.opt()], outs=[kgb.opt()])
                nc.gpsimd.collective_compute(
                    "AllGather", mybir.AluOpType.bypass, replica_groups=PAIRS,
                    ins=[kT8_own[0].opt()], outs=[kg8[0].opt()])

                # --- K c_hi rows: fp8 DoubleRow
                for m in range(DC):
                    ps = ps_all.tile([P, 512], F32, tag="ps")
                    for t in range(8):
                        nc.tensor.matmul(
                            ps[:], lhsT=wk8[m][:, ds(2 * t, 2), :],
                            rhs=xt8[:, ds(2 * t, 2), :],
                            start=(t == 0), stop=(t == 7), perf_mode=DR,
                        )
                    st8 = st8_pool.tile([P, 512], F8, tag="st8")
                    nc.scalar.activation(st8[:], ps[:], Copy, scale=1.0 / WS)
                    nc.gpsimd.dma_start(kT8_own[1][:, m, :, :], st8[:])
                nc.gpsimd.collective_compute(
                    "AllGather", mybir.AluOpType.bypass, replica_groups=PAIRS,
                    ins=[kT8_own[1].opt()], outs=[kg8[1].opt()])

                # --- V: n-outer so only 2 wv tile pairs are resident
                def load_wv(pool, dram, n, hb, dt, name):
                    wvt = pool.tile([P, 8, 512], dt, tag="wv", name=name)
                    nc.sync.dma_start(wvt[:], dram.ap()[n, hb])
                    return wvt

                wvb_t = {}
                wv8_t = {}
                for n in range(2):
                    for hb in range(2):
                        wvb_t[n, hb] = load_wv(wv_pool, wvb_d, n, hb, BF16,
                                               f"wvb{n}{hb}")
                        wv8_t[n, hb] = load_wv(wv8_pool, wv8_d, n, hb, F8,
                                               f"wv8{n}{hb}")

                # Q weight loads: emitted here so they stream during K/V
                wqb = [load_wb(wqb_d, m, f"wqb{m}") for m in range(DC)]
                wq8 = [load_w8(wq8_d, m, f"wq8{m}") for m in range(DC)]

                for n in range(4):
                    if n + 2 < 4:
                        for hb in range(2):
                            wvb_t[n + 2, hb] = load_wv(
                                wv_pool, wvb_d, n + 2, hb, BF16,
                                f"wvb{n + 2}{hb}")
                            wv8_t[n + 2, hb] = load_wv(
                                wv8_pool, wv8_d, n + 2, hb, F8,
                                f"wv8{n + 2}{hb}")
                    # c_lo rows: bf16, dual-staged
                    for s in range(4):
                        ps = ps_all.tile([P, 512], F32, tag="ps")
                        for dc in range(DC):
                            w = wvb_t[n, dc // 8]
                            nc.tensor.matmul(
                                ps[:], lhsT=xtb_ap(dc)[:, ts(s, P)],
                                rhs=w[:, dc % 8, :],
                                start=(dc == 0), stop=(dc == DC - 1),
                            )
                        svb = sv_pool.tile([P, 512], BF16, tag="sv")
                        nc.vector.tensor_copy(svb[:], ps[:])
                        sv8 = sv8_pool.tile([P, 512], F8, tag="sv8")
                        nc.scalar.copy(sv8[:], ps[:])
                        nc.gpsimd.dma_start(
                            vvb_own[s // 2][s % 2, :, ts(n, 512)], svb[:])
                        nc.gpsimd.dma_start(
                            vv8_own[s // 2][s % 2, :, ts(n, 512)], sv8[:])
                    # c_hi rows: fp8 DoubleRow
                    for s in range(4):
                        ps = ps_all.tile([P, 512], F32, tag="ps")
                        for t in range(8):
                            w = wv8_t[n, t // 4]
                            nc.tensor.matmul(
                                ps[:], lhsT=xt8[:, ds(2 * t, 2), ts(s, P)],
                                rhs=w[:, ds(2 * (t % 4), 2), :],
                                start=(t == 0), stop=(t == 7), perf_mode=DR,
                            )
                        sv8 = sv8_pool.tile([P, 512], F8, tag="sv8")
                        nc.scalar.activation(sv8[:], ps[:], Copy,
                                             scale=1.0 / WS)
                        nc.gpsimd.dma_start(
                            vv8_own[2 + s // 2][s % 2, :, ts(n, 512)], sv8[:])

                # --- Q: c_lo bf16 / c_hi fp8 DR, staged straight into the
                # phase-2 SBUF q tiles (no DRAM round-trip, no gpsimd dep:
                # the V AllGathers below can't stall Q)
                for m in range(DC):
                    ps = ps_all.tile([P, 512], F32, tag="ps")
                    for dc in range(DC):
                        nc.tensor.matmul(
                            ps[:], lhsT=wb_ap(wqb[m], dc), rhs=xtb_ap(dc),
                            start=(dc == 0), stop=(dc == DC - 1),
                        )
                    nc.scalar.copy(qtb[:, m, :], ps[:])
                for m in range(DC):
                    ps = ps_all.tile([P, 512], F32, tag="ps")
                    for t in range(8):
                        nc.tensor.matmul(
                            ps[:], lhsT=wq8[m][:, ds(2 * t, 2), :],
                            rhs=xt8[:, ds(2 * t, 2), :],
                            start=(t == 0), stop=(t == 7), perf_mode=DR,
                        )
                    nc.scalar.activation(qt8[:, m, :], ps[:], Copy,
                                         scale=1.0 / WS)

                for g in range(2):
                    nc.gpsimd.collective_compute(
                        "AllGather", mybir.AluOpType.bypass,
                        replica_groups=PAIRS,
                        ins=[vvb_own[g].opt()], outs=[vgb[g].opt()])
                for g in range(4):
                    nc.gpsimd.collective_compute(
                        "AllGather", mybir.AluOpType.bypass,
                        replica_groups=PAIRS,
                        ins=[vv8_own[g].opt()], outs=[vg8[g].opt()])

            # ---------------- phase 2: attention ----------------
            with (
                tc.tile_pool(name="pt", bufs=1) as pt_pool,
                tc.tile_pool(name="mk", bufs=1) as mk_pool,
                tc.tile_pool(name="vb", bufs=1) as vb_pool,
                tc.tile_pool(name="ktb", bufs=4) as ktb_pool,
                tc.tile_pool(name="kt8", bufs=8) as kt8_pool,
                tc.tile_pool(name="one", bufs=1) as one_pool,
                tc.tile_pool(name="sc", bufs=4) as sc_pool,
                tc.tile_pool(name="ob", bufs=4) as ob_pool,
            ):
                mkb = mk_pool.tile([P, 8, 512], BF16, tag="mkb")
                nc.sync.dma_start(mkb[:, :4, :], mkb_d.ap()[:, :4, :])
                nc.sync.dma_start(mkb[:, 4:, :], mkb_d.ap()[:, 4:, :])
                mk8 = mk_pool.tile([P, 16, 512], F8, tag="mk8")
                nc.sync.dma_start(mk8[:, :8, :], mk8_d.ap()[:, :8, :])
                nc.sync.dma_start(mk8[:, 8:, :], mk8_d.ap()[:, 8:, :])
                onesb = one_pool.tile([P, 1], BF16, tag="onesb")
                nc.vector.memset(onesb[:], 1.0)
                ones8 = one_pool.tile([P, 2, 1], F8, tag="ones8")
                nc.vector.memset(ones8[:], 1.0)
                ebias = one_pool.tile([P, 1], F32, tag="ebias")
                nc.vector.memset(ebias[:], ESHIFT)

                ptb = pt_pool.tile([P, 8, 512], BF16, tag="ptb")
                pt8 = pt_pool.tile([P, 16, 512], F8, tag="pt8")

                # V big tiles: emitted before the score loop so the 8MB
                # streams in under the score matmuls
                vbb = vb_pool.tile([P, 8, D], BF16, tag="vbb")
                for jj, pos in enumerate(SLOT0_POS):
                    nc.sync.dma_start(
                        vbb[:, jj, :],
                        vgb[(pos % 8) // 2][pos // 8, pos % 2])
                v8b = vb_pool.tile([P, 16, D], F8, tag="v8b")
                for pos in range(SB):
                    nc.sync.dma_start(
                        v8b[:, pos, :],
                        vg8[(pos % 8) // 2][pos // 8, pos % 2])

                def kt8_src(pos):
                    lo = pos in (0, 1, 2, 3, 8, 9, 10, 11)
                    return kg8[0 if lo else 1][pos // 8][:, :, pos % 4, :]

                for pos in range(SB):
                    kt8_t = kt8_pool.tile([P, DC, P], F8, tag="kt8",
                                          name=f"kt8_{pos}")
                    nc.sync.dma_start(kt8_t[:], kt8_src(pos))
                    ps = ps_all.tile([P, 512], F32, tag="ps")
                    for t in range(8):
                        nc.tensor.matmul(
                            ps[:], lhsT=kt8_t[:, ds(2 * t, 2), :],
                            rhs=qt8[:, ds(2 * t, 2), :],
                            start=(t == 0), stop=(t == 7), perf_mode=DR,
                        )
                    nc.scalar.activation(pt8[:, pos, :], ps[:], Exp,
                                         scale=INV_SQRT_D, bias=ebias[:])
                    if pos in S0IDX:
                        ktb_t = ktb_pool.tile([P, DC, P], BF16, tag="ktb",
                                              name=f"ktb_{pos}")
                        nc.sync.dma_start(
                            ktb_t[:], kgb[pos // 8][:, :, pos % 4, :])
                        ps2 = ps_all.tile([P, 512], F32, tag="ps")
                        for dc in range(DC):
                            nc.tensor.matmul(
                                ps2[:], lhsT=ktb_t[:, dc, :],
                                rhs=qtb[:, dc, :],
                                start=(dc == 0), stop=(dc == DC - 1),
                            )
                        nc.scalar.activation(ptb[:, S0IDX[pos], :], ps2[:],
                                             Exp, scale=INV_SQRT_D)

                for j in range(8):
                    nc.vector.tensor_mul(ptb[:, j, :], ptb[:, j, :],
                                         mkb[:, j, :])
                for pos in range(SB):
                    nc.vector.tensor_mul(pt8[:, pos, :], pt8[:, pos, :],
                                         mk8[:, pos, :])

                # --- slot 0 (bf16): rowsum, AV, normalize, out rows 0..511
                for qs in range(4):
                    kpos = [0, 1, 2, 3] + list(range(8, 9 + qs))
                    idx = [S0IDX[p] for p in kpos]
                    plt = ps_all.tile([P, 512], F32, tag="ps", name="pl")
                    pl = plt[:, :1]
                    for i, j in enumerate(idx):
                        nc.tensor.matmul(
                            pl[:], lhsT=ptb[:, j, ts(qs, P)], rhs=onesb[:],
                            start=(i == 0), stop=(i == len(idx) - 1),
                        )
                    rl = sc_pool.tile([P, 1], F32, tag="rl")
                    nc.vector.reciprocal(rl[:], pl[:])
                    for n in range(4):
                        pav = ps_all.tile([P, 512], F32, tag="ps", name="pav")
                        for i, j in enumerate(idx):
                            nc.tensor.matmul(
                                pav[:], lhsT=ptb[:, j, ts(qs, P)],
                                rhs=vbb[:, j, ts(n, 512)],
                                start=(i == 0), stop=(i == len(idx) - 1),
                            )
                        ob = ob_pool.tile([P, 512], F32, tag="ob")
                        if n % 2 == 0:
                            nc.vector.tensor_scalar_mul(ob[:], pav[:], rl[:])
                        else:
                            nc.scalar.activation(ob[:], pav[:], Copy,
                                                 scale=rl[:])
                        nc.sync.dma_start(
                            out_d.ap()[ds(qs * P, P), ts(n, 512)], ob[:])

                # --- slot 1 (fp8 DR): rows 512..1023
                for qs in range(4):
                    run1 = 6 if qs < 2 else 8   # [0..4+qs] rounded to even
                    pstarts = list(range(0, run1, 2)) + [8, 10, 12, 14]
                    plt = ps_all.tile([P, 512], F32, tag="ps", name="pl8")
                    pl = plt[:, :1]
                    for i, p0 in enumerate(pstarts):
                        nc.tensor.matmul(
                            pl[:], lhsT=pt8[:, ds(p0, 2), ts(qs, P)],
                            rhs=ones8[:],
                            start=(i == 0), stop=(i == len(pstarts) - 1),
                            perf_mode=DR,
                        )
                    rl = sc_pool.tile([P, 1], F32, tag="rl")
                    nc.vector.reciprocal(rl[:], pl[:])
                    for n in range(4):
                        pav = ps_all.tile([P, 512], F32, tag="ps", name="pav8")
                        for i, p0 in enumerate(pstarts):
                            nc.tensor.matmul(
                                pav[:], lhsT=pt8[:, ds(p0, 2), ts(qs, P)],
                                rhs=v8b[:, ds(p0, 2), ts(n, 512)],
                                start=(i == 0), stop=(i == len(pstarts) - 1),
                                perf_mode=DR,
                            )
                        ob = ob_pool.tile([P, 512], F32, tag="ob")
                        if n % 2 == 0:
                            nc.vector.tensor_scalar_mul(ob[:], pav[:], rl[:])
                        else:
                            nc.scalar.activation(ob[:], pav[:], Copy,
                                                 scale=rl[:])
                        nc.sync.dma_start(
                            out_d.ap()[ds(512 + qs * P, P), ts(n, 512)], ob[:])

    nc.compile()
    _CACHED_NC = nc
    return nc


def _host_prep(x, Wq, Wk, Wv):
    """Build per-core input maps (host-side layout prep)."""
    f8 = ml_dtypes.float8_e4m3
    bf = ml_dtypes.bfloat16

    def wqk_layout(W, dt, scale=1.0):
        return np.ascontiguousarray(
            (W * scale).reshape(DC, P, DC, P).transpose(2, 1, 0, 3)).astype(dt)

    def wv_layout(W, dt, scale=1.0):
        return np.ascontiguousarray(
            (W * scale).reshape(2, 8, P, 4, 512).transpose(3, 0, 2, 1, 4)
        ).astype(dt)

    wqb_h = wqk_layout(Wq, bf)
    wq8_h = wqk_layout(Wq, f8, WS)
    wkb_h = wqk_layout(Wk, bf)
    wk8_h = wqk_layout(Wk, f8, WS)
    wvb_h = wv_layout(Wv, bf)
    wv8_h = wv_layout(Wv, f8, WS)

    k_in_block = np.arange(P, dtype=np.int64)[:, None]           # [P, 1]
    q_in_chunk = np.arange(512, dtype=np.int64)[None, :]         # [1, 512]

    def build_masks(h):
        c_lo, c_hi = h, 3 - h
        mb = np.zeros((P, 8, 512), dtype=bf)
        for j, pos in enumerate(SLOT0_POS):
            tkb = POS2TRUE[pos // 4] * 4 + pos % 4
            mb[:, j, :] = (tkb * P + k_in_block) <= (c_lo * 512 + q_in_chunk)
        m8 = np.zeros((P, 16, 512), dtype=f8)
        for pos in range(SB):
            tkb = POS2TRUE[pos // 4] * 4 + pos % 4
            m8[:, pos, :] = (tkb * P + k_in_block) <= (c_hi * 512 + q_in_chunk)
        return mb, m8

    mask_h = [build_masks(0), build_masks(1)]

    in_maps = []
    for core in range(NCORES):
        b, h = divmod(core, 2)
        c_lo, c_hi = h, 3 - h
        xt = x[b].T                                               # [D, S] view
        xtb_h = np.ascontiguousarray(
            xt[:, c_lo * 512:(c_lo + 1) * 512].reshape(DC, P, 512)
            .transpose(1, 0, 2)).astype(bf)
        xt8_h = np.ascontiguousarray(
            xt[:, c_hi * 512:(c_hi + 1) * 512].reshape(DC, P, 512)
            .transpose(1, 0, 2)).astype(f8)
        in_maps.append({
            "xtb": xtb_h,
            "xt8": xt8_h,
            "wqb": wqb_h, "wq8": wq8_h, "wkb": wkb_h, "wk8": wk8_h,
            "wvb": wvb_h, "wv8": wv8_h,
            "maskb": mask_h[h][0], "mask8": mask_h[h][1],
        })
    return in_maps


def run(x, Wq, Wk, Wv, trace=False):
    x = np.asarray(x, dtype=np.float32)
    Wq = np.asarray(Wq, dtype=np.float32)
    Wk = np.asarray(Wk, dtype=np.float32)
    Wv = np.asarray(Wv, dtype=np.float32)
    nc = build_nc()
    in_maps = _host_prep(x, Wq, Wk, Wv)
    res = run_bass_kernel_spmd(nc, in_maps, core_ids=list(range(NCORES)),
                               trace=trace)
    out = np.empty((B, S, D), dtype=np.float32)
    for core in range(NCORES):
        b, h = divmod(core, 2)
        c_lo, c_hi = h, 3 - h
        o = res.results[core]["out"]
        out[b, c_lo * 512:(c_lo + 1) * 512] = o[:512]
        out[b, c_hi * 512:(c_hi + 1) * 512] = o[512:]
    return out, res


def kernel(x, Wq, Wk, Wv):
    out, _ = run(x, Wq, Wk, Wv)
    return out


if __name__ == "__main__":
    build_nc()
    print("build + compile OK")


# revision 29
# speedup vs baseline: 1.2176x; 1.0019x over previous
"""Causal attention kernel for Trainium2, 8 NeuronCores — depth-split fp8.

Problem: x[4,2048,2048] @ Wq/Wk/Wv[2048,2048] -> causal softmax attention.

Sharding (as baseline): 2 cores per batch; each core owns 1024 query rows as
global 512-row chunks {0,3} (even cores) / {1,2} (odd cores). Pairwise
AllGather assembles full K^T / V per batch. Gathered key-block positions are
[chunk0, chunk3, chunk1, chunk2] on every core; query slot 0 (c_lo) attends
positions {0-3, 8-11}, slot 1 (c_hi) all 16; causality via mask tensors.

Depth-split fp8: rows in chunks 2,3 (slot 1, >=1025 visible keys) have
diffuse softmax, so fp8-e4m3 noise (~4%/tensor) attenuates to <1% there:
  - slot-1 Q/K/V projections in fp8 DoubleRow (2 fp8/PE cell, 256-deep
    contraction per pass), weights host-scaled x64, staged back at 1/64.
  - slot-1 scores fp8 DoubleRow; exp shifted by -2 so unnormalized fp8
    probs stay < 240 (TRN e4m3 max); shift cancels in normalization.
  - slot-1 AV in fp8 DoubleRow (adjacent key-block pairs; odd-length
    causal runs rounded up — the padded block's probs are mask-zeroed).
  - slot-0 (chunks 0,1) stays bf16 end-to-end: shallow rows concentrate
    softmax mass and pass v through nearly verbatim (sim: bf16 4.9e-3,
    full-fp8 4.5e-2, this split 1.33e-2 vs the 2e-2 gate).
c_lo K/V are computed in bf16 and dual-staged (bf16 + fp8 copy); c_hi rows
never need bf16 x.

Scheduling: staging stores ride the gpsimd queue (engine-local DIRECT2D);
the 9 collectives are emitted so nothing tensor-critical queues behind
their peer-sync waits — in particular Q staging writes PSUM->SBUF straight
into the phase-2 q tiles (no DRAM round-trip, no gpsimd dependency).
First x/W tiles are split small so the first matmul waits on ~0.75MB.
"""

import math

import numpy as np
import ml_dtypes

import concourse.bass as bass
import concourse.mybir as mybir
import concourse.tile as tile
from concourse import bacc
from concourse.bass import ds, ts
from concourse.bass_utils import run_bass_kernel_spmd

B, S, D = 4, 2048, 2048
P = 128
DC = D // P          # 16 contraction chunks
SB = S // P          # 16 key blocks
QROWS = 1024         # query rows per core
NCORES = 8
INV_SQRT_D = 1.0 / math.sqrt(D)
WS = 64.0            # host scale on fp8 weights (keeps W8 in e4m3 normal range)
ESHIFT = -2.0        # exp shift: unnormalized fp8 probs < 240

# gathered key-block position -> true 512-chunk (pair-rank order, all cores)
POS2TRUE = [0, 3, 1, 2]
SLOT0_POS = [0, 1, 2, 3, 8, 9, 10, 11]   # slot-0's (bf16) key positions
S0IDX = {pos: j for j, pos in enumerate(SLOT0_POS)}
PAIRS = [[0, 1], [2, 3], [4, 5], [6, 7]]

F32 = mybir.dt.float32
BF16 = mybir.dt.bfloat16
F8 = mybir.dt.float8e4
DR = mybir.MatmulPerfMode.DoubleRow
Exp = mybir.ActivationFunctionType.Exp
Copy = mybir.ActivationFunctionType.Copy

_CACHED_NC = None


def build_nc():
    global _CACHED_NC
    if _CACHED_NC is not None:
        return _CACHED_NC
    nc = bacc.Bacc(trn_type="TRN2", target_bir_lowering=False, debug=False,
                   num_devices=NCORES)

    # x^T shipped pre-tiled as [P, DC, 512] so SBUF loads are single DMAs
    xtb_d = nc.dram_tensor("xtb", [P, DC, 512], BF16, kind="ExternalInput")
    xt8_d = nc.dram_tensor("xt8", [P, DC, 512], F8, kind="ExternalInput")
    wqb_d = nc.dram_tensor("wqb", [DC, P, DC, P], BF16, kind="ExternalInput")
    wq8_d = nc.dram_tensor("wq8", [DC, P, DC, P], F8, kind="ExternalInput")
    wkb_d = nc.dram_tensor("wkb", [DC, P, DC, P], BF16, kind="ExternalInput")
    wk8_d = nc.dram_tensor("wk8", [DC, P, DC, P], F8, kind="ExternalInput")
    wvb_d = nc.dram_tensor("wvb", [4, 2, P, 8, 512], BF16, kind="ExternalInput")
    wv8_d = nc.dram_tensor("wv8", [4, 2, P, 8, 512], F8, kind="ExternalInput")
    mkb_d = nc.dram_tensor("maskb", [P, 8, 512], BF16, kind="ExternalInput")
    mk8_d = nc.dram_tensor("mask8", [P, 16, 512], F8, kind="ExternalInput")
    out_d = nc.dram_tensor("out", [QROWS, D], F32, kind="ExternalOutput")

    with tile.TileContext(nc) as tc:
        with (
            tc.tile_pool(name="dram", bufs=1, space="DRAM") as dpool,
            tc.tile_pool(name="ps", bufs=8, space="PSUM") as ps_all,
            tc.tile_pool(name="qsb", bufs=1) as qsb_pool,
        ):
            # phase-2 q tiles, written directly by Q staging (PSUM->SBUF)
            qtb = qsb_pool.tile([P, DC, 512], BF16, tag="qtb")
            qt8 = qsb_pool.tile([P, DC, 512], F8, tag="qt8")

            # [P, DC, 4, P] layout: a whole [P,512] staging tile stores with
            # ONE dma, and a whole [P, DC, P] kt tile loads with one dma
            kTb_own = dpool.tile([P, DC, 4, P], BF16, tag="kTbo")
            kT8_own = [dpool.tile([P, DC, 4, P], F8, tag=f"kT8o{s}",
                                  name=f"kT8o{s}") for s in range(2)]
            kgb = dpool.tile([2, P, DC, 4, P], BF16, tag="kgb")
            kg8 = [dpool.tile([2, P, DC, 4, P], F8, tag=f"kg8{s}",
                              name=f"kg8{s}") for s in range(2)]
            vvb_own = [dpool.tile([2, P, D], BF16, tag=f"vvbo{g}",
                                  name=f"vvbo{g}") for g in range(2)]
            vv8_own = [dpool.tile([2, P, D], F8, tag=f"vv8o{g}",
                                  name=f"vv8o{g}") for g in range(4)]
            vgb = [dpool.tile([2, 2, P, D], BF16, tag=f"vgb{g}",
                              name=f"vgb{g}") for g in range(2)]
            vg8 = [dpool.tile([2, 2, P, D], F8, tag=f"vg8{g}",
                              name=f"vg8{g}") for g in range(4)]

            # ---------------- phase 1: projections ----------------
            with (
                tc.tile_pool(name="xt", bufs=1) as xt_pool,
                tc.tile_pool(name="wbf", bufs=20) as wbf_pool,
                tc.tile_pool(name="w8", bufs=16) as w8_pool,
                tc.tile_pool(name="wv", bufs=5) as wv_pool,
                tc.tile_pool(name="wv8", bufs=4) as wv8_pool,
                tc.tile_pool(name="st", bufs=8) as st_pool,
                tc.tile_pool(name="st8", bufs=8) as st8_pool,
                tc.tile_pool(name="sv", bufs=5) as sv_pool,
                tc.tile_pool(name="sv8", bufs=6) as sv8_pool,
            ):
                # bf16 W tiles split in half so the first matmul's deps
                # are small (dep tracking is tile-granular)
                def load_wb(dram, m, name):
                    lo = wbf_pool.tile([P, 8, P], BF16, tag="w",
                                       name=f"{name}l")
                    nc.sync.dma_start(lo[:], dram.ap()[m][:, :8, :])
                    hi = wbf_pool.tile([P, 8, P], BF16, tag="w",
                                       name=f"{name}h")
                    nc.sync.dma_start(hi[:], dram.ap()[m][:, 8:, :])
                    return lo, hi

                def wb_ap(wpair, dc):
                    return wpair[dc // 8][:, dc % 8, :]

                def load_w8(dram, m, name):
                    wt = w8_pool.tile([P, DC, P], F8, tag="w", name=name)
                    nc.sync.dma_start(wt[:], dram.ap()[m])
                    return wt

                wkb = [load_wb(wkb_d, 0, "wkb0")]
                # x^T c_lo split into 4 tiles so the first matmuls' deps
                # are 0.5MB each
                xtbs = []
                for c in range(4):
                    t = xt_pool.tile([P, 4, 512], BF16, tag=f"xtb{c}",
                                     name=f"xtb{c}")
                    nc.sync.dma_start(t[:], xtb_d.ap()[:, ds(4 * c, 4), :])
                    xtbs.append(t)

                def xtb_ap(dc):
                    return xtbs[dc // 4][:, dc % 4, :]

                wkb += [load_wb(wkb_d, m, f"wkb{m}") for m in range(1, 8)]
                xt8 = xt_pool.tile([P, DC, 512], F8, tag="xt8")
                nc.sync.dma_start(xt8[:, :8, :], xt8_d.ap()[:, :8, :])
                nc.sync.dma_start(xt8[:, 8:, :], xt8_d.ap()[:, 8:, :])
                wkb += [load_wb(wkb_d, m, f"wkb{m}") for m in range(8, DC)]
                wk8 = [load_w8(wk8_d, m, f"wk8{m}") for m in range(DC)]

                # --- K c_lo rows: bf16, dual-staged (bf16 + fp8)
                for m in range(DC):
                    ps = ps_all.tile([P, 512], F32, tag="ps")
                    for dc in range(DC):
                        nc.tensor.matmul(
                            ps[:], lhsT=wb_ap(wkb[m], dc), rhs=xtb_ap(dc),
                            start=(dc == 0), stop=(dc == DC - 1),
                        )
                    stb = st_pool.tile([P, 512], BF16, tag="st")
                    nc.scalar.copy(stb[:], ps[:])
                    st8 = st8_pool.tile([P, 512], F8, tag="st8")
                    nc.vector.tensor_copy(st8[:], ps[:])
                    nc.scalar.dma_start(kTb_own[:, m, :, :], stb[:])
                    nc.scalar.dma_start(kT8_own[0][:, m, :, :], st8[:])
                nc.gpsimd.collective_compute(
                    "AllGather", mybir.AluOpType.bypass, replica_groups=PAIRS,
                    ins=[kTb_own.opt()], outs=[kgb.opt()])
                nc.gpsimd.collective_compute(
                    "AllGather", mybir.AluOpType.bypass, replica_groups=PAIRS,
                    ins=[kT8_own[0].opt()], outs=[kg8[0].opt()])

                # --- K c_hi rows: fp8 DoubleRow
                for m in range(DC):
                    ps = ps_all.tile([P, 512], F32, tag="ps")
                    for t in range(8):
                        nc.tensor.matmul(
                            ps[:], lhsT=wk8[m][:, ds(2 * t, 2), :],
                            rhs=xt8[:, ds(2 * t, 2), :],
                            start=(t == 0), stop=(t == 7), perf_mode=DR,
                        )
                    st8 = st8_pool.tile([P, 512], F8, tag="st8")
                    nc.scalar.activation(st8[:], ps[:], Copy, scale=1.0 / WS)
                    nc.scalar.dma_start(kT8_own[1][:, m, :, :], st8[:])
                nc.gpsimd.collective_compute(
                    "AllGather", mybir.AluOpType.bypass, replica_groups=PAIRS,
                    ins=[kT8_own[1].opt()], outs=[kg8[1].opt()])

                # --- V: n-outer so only 2 wv tile pairs are resident
                def load_wv(pool, dram, n, hb, dt, name):
                    wvt = pool.tile([P, 8, 512], dt, tag="wv", name=name)
                    nc.sync.dma_start(wvt[:], dram.ap()[n, hb])
                    return wvt

                wvb_t = {}
                wv8_t = {}
                for n in range(2):
                    for hb in range(2):
                        wvb_t[n, hb] = load_wv(wv_pool, wvb_d, n, hb, BF16,
                                               f"wvb{n}{hb}")
                        wv8_t[n, hb] = load_wv(wv8_pool, wv8_d, n, hb, F8,
                                               f"wv8{n}{hb}")

                # Q weight loads: emitted here so they stream during K/V
                wqb = [load_wb(wqb_d, m, f"wqb{m}") for m in range(DC)]
                wq8 = [load_w8(wq8_d, m, f"wq8{m}") for m in range(DC)]

                for n in range(4):
                    if n + 2 < 4:
                        for hb in range(2):
                            wvb_t[n + 2, hb] = load_wv(
                                wv_pool, wvb_d, n + 2, hb, BF16,
                                f"wvb{n + 2}{hb}")
                            wv8_t[n + 2, hb] = load_wv(
                                wv8_pool, wv8_d, n + 2, hb, F8,
                                f"wv8{n + 2}{hb}")
                    # c_lo rows: bf16, dual-staged
                    for s in range(4):
                        ps = ps_all.tile([P, 512], F32, tag="ps")
                        for dc in range(DC):
                            w = wvb_t[n, dc // 8]
                            nc.tensor.matmul(
                                ps[:], lhsT=xtb_ap(dc)[:, ts(s, P)],
                                rhs=w[:, dc % 8, :],
                                start=(dc == 0), stop=(dc == DC - 1),
                            )
                        svb = sv_pool.tile([P, 512], BF16, tag="sv")
                        nc.vector.tensor_copy(svb[:], ps[:])
                        sv8 = sv8_pool.tile([P, 512], F8, tag="sv8")
                        nc.scalar.copy(sv8[:], ps[:])
                        nc.scalar.dma_start(
                            vvb_own[s // 2][s % 2, :, ts(n, 512)], svb[:])
                        nc.scalar.dma_start(
                            vv8_own[s // 2][s % 2, :, ts(n, 512)], sv8[:])
                    # c_hi rows: fp8 DoubleRow
                    for s in range(4):
                        ps = ps_all.tile([P, 512], F32, tag="ps")
                        for t in range(8):
                            w = wv8_t[n, t // 4]
                            nc.tensor.matmul(
                                ps[:], lhsT=xt8[:, ds(2 * t, 2), ts(s, P)],
                                rhs=w[:, ds(2 * (t % 4), 2), :],
                                start=(t == 0), stop=(t == 7), perf_mode=DR,
                            )
                        sv8 = sv8_pool.tile([P, 512], F8, tag="sv8")
                        nc.scalar.activation(sv8[:], ps[:], Copy,
                                             scale=1.0 / WS)
                        nc.scalar.dma_start(
                            vv8_own[2 + s // 2][s % 2, :, ts(n, 512)], sv8[:])

                # --- Q: c_lo bf16 / c_hi fp8 DR, staged straight into the
                # phase-2 SBUF q tiles (no DRAM round-trip, no gpsimd dep:
                # the V AllGathers below can't stall Q)
                for m in range(DC):
                    ps = ps_all.tile([P, 512], F32, tag="ps")
                    for dc in range(DC):
                        nc.tensor.matmul(
                            ps[:], lhsT=wb_ap(wqb[m], dc), rhs=xtb_ap(dc),
                            start=(dc == 0), stop=(dc == DC - 1),
                        )
                    nc.scalar.copy(qtb[:, m, :], ps[:])
                for m in range(DC):
                    ps = ps_all.tile([P, 512], F32, tag="ps")
                    for t in range(8):
                        nc.tensor.matmul(
                            ps[:], lhsT=wq8[m][:, ds(2 * t, 2), :],
                            rhs=xt8[:, ds(2 * t, 2), :],
                            start=(t == 0), stop=(t == 7), perf_mode=DR,
                        )
                    nc.scalar.activation(qt8[:, m, :], ps[:], Copy,
                                         scale=1.0 / WS)

                for g in range(2):
                    nc.gpsimd.collective_compute(
                        "AllGather", mybir.AluOpType.bypass,
                        replica_groups=PAIRS,
                        ins=[vvb_own[g].opt()], outs=[vgb[g].opt()])
                for g in range(4):
                    nc.gpsimd.collective_compute(
                        "AllGather", mybir.AluOpType.bypass,
                        replica_groups=PAIRS,
                        ins=[vv8_own[g].opt()], outs=[vg8[g].opt()])

            # ---------------- phase 2: attention ----------------
            with (
                tc.tile_pool(name="pt", bufs=1) as pt_pool,
                tc.tile_pool(name="mk", bufs=1) as mk_pool,
                tc.tile_pool(name="vb", bufs=1) as vb_pool,
                tc.tile_pool(name="ktb", bufs=8) as ktb_pool,
                tc.tile_pool(name="kt8", bufs=16) as kt8_pool,
                tc.tile_pool(name="one", bufs=1) as one_pool,
                tc.tile_pool(name="sc", bufs=4) as sc_pool,
                tc.tile_pool(name="ob", bufs=4) as ob_pool,
            ):
                mkb = mk_pool.tile([P, 8, 512], BF16, tag="mkb")
                nc.sync.dma_start(mkb[:, :4, :], mkb_d.ap()[:, :4, :])
                nc.sync.dma_start(mkb[:, 4:, :], mkb_d.ap()[:, 4:, :])
                mk8 = mk_pool.tile([P, 16, 512], F8, tag="mk8")
                nc.sync.dma_start(mk8[:, :8, :], mk8_d.ap()[:, :8, :])
                nc.sync.dma_start(mk8[:, 8:, :], mk8_d.ap()[:, 8:, :])
                onesb = one_pool.tile([P, 1], BF16, tag="onesb")
                nc.vector.memset(onesb[:], 1.0)
                ones8 = one_pool.tile([P, 2, 1], F8, tag="ones8")
                nc.vector.memset(ones8[:], 1.0)
                ebias = one_pool.tile([P, 1], F32, tag="ebias")
                nc.vector.memset(ebias[:], ESHIFT)

                ptb = pt_pool.tile([P, 8, 512], BF16, tag="ptb")
                pt8 = pt_pool.tile([P, 16, 512], F8, tag="pt8")

                # all kt tiles preloaded (fully resident, single-dma each),
                # then the V big tiles — kt loads must dispatch first so
                # score matmuls aren't gated by vbig's gather waits
                def kt8_src(pos):
                    lo = pos in (0, 1, 2, 3, 8, 9, 10, 11)
                    return kg8[0 if lo else 1][pos // 8][:, :, pos % 4, :]

                kt8_ts = []
                ktb_ts = {}
                for pos in range(SB):
                    kt8_t = kt8_pool.tile([P, DC, P], F8, tag="kt8",
                                          name=f"kt8_{pos}")
                    nc.sync.dma_start(kt8_t[:], kt8_src(pos))
                    kt8_ts.append(kt8_t)
                    if pos in S0IDX:
                        ktb_t = ktb_pool.tile([P, DC, P], BF16, tag="ktb",
                                              name=f"ktb_{pos}")
                        nc.sync.dma_start(
                            ktb_t[:], kgb[pos // 8][:, :, pos % 4, :])
                        ktb_ts[pos] = ktb_t

                vbb = vb_pool.tile([P, 8, D], BF16, tag="vbb")
                for jj, pos in enumerate(SLOT0_POS):
                    nc.sync.dma_start(
                        vbb[:, jj, :],
                        vgb[(pos % 8) // 2][pos // 8, pos % 2])
                v8b = vb_pool.tile([P, 16, D], F8, tag="v8b")
                for pos in range(SB):
                    nc.sync.dma_start(
                        v8b[:, pos, :],
                        vg8[(pos % 8) // 2][pos // 8, pos % 2])

                for pos in range(SB):
                    kt8_t = kt8_ts[pos]
                    ps = ps_all.tile([P, 512], F32, tag="ps")
                    for t in range(8):
                        nc.tensor.matmul(
                            ps[:], lhsT=kt8_t[:, ds(2 * t, 2), :],
                            rhs=qt8[:, ds(2 * t, 2), :],
                            start=(t == 0), stop=(t == 7), perf_mode=DR,
                        )
                    nc.scalar.activation(pt8[:, pos, :], ps[:], Exp,
                                         scale=INV_SQRT_D, bias=ebias[:])
                    if pos in S0IDX:
                        ktb_t = ktb_ts[pos]
                        ps2 = ps_all.tile([P, 512], F32, tag="ps")
                        for dc in range(DC):
                            nc.tensor.matmul(
                                ps2[:], lhsT=ktb_t[:, dc, :],
                                rhs=qtb[:, dc, :],
                                start=(dc == 0), stop=(dc == DC - 1),
                            )
                        nc.scalar.activation(ptb[:, S0IDX[pos], :], ps2[:],
                                             Exp, scale=INV_SQRT_D)

                for j in range(8):
                    nc.vector.tensor_mul(ptb[:, j, :], ptb[:, j, :],
                                         mkb[:, j, :])
                for pos in range(SB):
                    nc.vector.tensor_mul(pt8[:, pos, :], pt8[:, pos, :],
                                         mk8[:, pos, :])

                # --- slot 0 (bf16): rowsum, AV, normalize, out rows 0..511
                for qs in range(4):
                    kpos = [0, 1, 2, 3] + list(range(8, 9 + qs))
                    idx = [S0IDX[p] for p in kpos]
                    plt = ps_all.tile([P, 512], F32, tag="ps", name="pl")
                    pl = plt[:, :1]
                    for i, j in enumerate(idx):
                        nc.tensor.matmul(
                            pl[:], lhsT=ptb[:, j, ts(qs, P)], rhs=onesb[:],
                            start=(i == 0), stop=(i == len(idx) - 1),
                        )
                    rl = sc_pool.tile([P, 1], F32, tag="rl")
                    nc.vector.reciprocal(rl[:], pl[:])
                    for n in range(4):
                        pav = ps_all.tile([P, 512], F32, tag="ps", name="pav")
                        for i, j in enumerate(idx):
                            nc.tensor.matmul(
                                pav[:], lhsT=ptb[:, j, ts(qs, P)],
                                rhs=vbb[:, j, ts(n, 512)],
                                start=(i == 0), stop=(i == len(idx) - 1),
                            )
                        ob = ob_pool.tile([P, 512], F32, tag="ob")
                        if n % 2 == 0:
                            nc.vector.tensor_scalar_mul(ob[:], pav[:], rl[:])
                        else:
                            nc.scalar.activation(ob[:], pav[:], Copy,
                                                 scale=rl[:])
                        nc.sync.dma_start(
                            out_d.ap()[ds(qs * P, P), ts(n, 512)], ob[:])

                # --- slot 1 (fp8 DR): rows 512..1023
                for qs in range(4):
                    run1 = 6 if qs < 2 else 8   # [0..4+qs] rounded to even
                    pstarts = list(range(0, run1, 2)) + [8, 10, 12, 14]
                    plt = ps_all.tile([P, 512], F32, tag="ps", name="pl8")
                    pl = plt[:, :1]
                    for i, p0 in enumerate(pstarts):
                        nc.tensor.matmul(
                            pl[:], lhsT=pt8[:, ds(p0, 2), ts(qs, P)],
                            rhs=ones8[:],
                            start=(i == 0), stop=(i == len(pstarts) - 1),
                            perf_mode=DR,
                        )
                    rl = sc_pool.tile([P, 1], F32, tag="rl")
                    nc.vector.reciprocal(rl[:], pl[:])
                    for n in range(4):
                        pav = ps_all.tile([P, 512], F32, tag="ps", name="pav8")
                        for i, p0 in enumerate(pstarts):
                            nc.tensor.matmul(
                                pav[:], lhsT=pt8[:, ds(p0, 2), ts(qs, P)],
                                rhs=v8b[:, ds(p0, 2), ts(n, 512)],
                                start=(i == 0), stop=(i == len(pstarts) - 1),
                                perf_mode=DR,
                            )
                        ob = ob_pool.tile([P, 512], F32, tag="ob")
                        if n % 2 == 0:
                            nc.vector.tensor_scalar_mul(ob[:], pav[:], rl[:])
                        else:
                            nc.scalar.activation(ob[:], pav[:], Copy,
                                                 scale=rl[:])
                        nc.sync.dma_start(
                            out_d.ap()[ds(512 + qs * P, P), ts(n, 512)], ob[:])

    nc.compile()
    _CACHED_NC = nc
    return nc


def _host_prep(x, Wq, Wk, Wv):
    """Build per-core input maps (host-side layout prep)."""
    f8 = ml_dtypes.float8_e4m3
    bf = ml_dtypes.bfloat16

    def wqk_layout(W, dt, scale=1.0):
        return np.ascontiguousarray(
            (W * scale).reshape(DC, P, DC, P).transpose(2, 1, 0, 3)).astype(dt)

    def wv_layout(W, dt, scale=1.0):
        return np.ascontiguousarray(
            (W * scale).reshape(2, 8, P, 4, 512).transpose(3, 0, 2, 1, 4)
        ).astype(dt)

    wqb_h = wqk_layout(Wq, bf)
    wq8_h = wqk_layout(Wq, f8, WS)
    wkb_h = wqk_layout(Wk, bf)
    wk8_h = wqk_layout(Wk, f8, WS)
    wvb_h = wv_layout(Wv, bf)
    wv8_h = wv_layout(Wv, f8, WS)

    k_in_block = np.arange(P, dtype=np.int64)[:, None]           # [P, 1]
    q_in_chunk = np.arange(512, dtype=np.int64)[None, :]         # [1, 512]

    def build_masks(h):
        c_lo, c_hi = h, 3 - h
        mb = np.zeros((P, 8, 512), dtype=bf)
        for j, pos in enumerate(SLOT0_POS):
            tkb = POS2TRUE[pos // 4] * 4 + pos % 4
            mb[:, j, :] = (tkb * P + k_in_block) <= (c_lo * 512 + q_in_chunk)
        m8 = np.zeros((P, 16, 512), dtype=f8)
        for pos in range(SB):
            tkb = POS2TRUE[pos // 4] * 4 + pos % 4
            m8[:, pos, :] = (tkb * P + k_in_block) <= (c_hi * 512 + q_in_chunk)
        return mb, m8

    mask_h = [build_masks(0), build_masks(1)]

    in_maps = []
    for core in range(NCORES):
        b, h = divmod(core, 2)
        c_lo, c_hi = h, 3 - h
        xt = x[b].T                                               # [D, S] view
        xtb_h = np.ascontiguousarray(
            xt[:, c_lo * 512:(c_lo + 1) * 512].reshape(DC, P, 512)
            .transpose(1, 0, 2)).astype(bf)
        xt8_h = np.ascontiguousarray(
            xt[:, c_hi * 512:(c_hi + 1) * 512].reshape(DC, P, 512)
            .transpose(1, 0, 2)).astype(f8)
        in_maps.append({
            "xtb": xtb_h,
            "xt8": xt8_h,
            "wqb": wqb_h, "wq8": wq8_h, "wkb": wkb_h, "wk8": wk8_h,
            "wvb": wvb_h, "wv8": wv8_h,
            "maskb": mask_h[h][0], "mask8": mask_h[h][1],
        })
    return in_maps


def run(x, Wq, Wk, Wv, trace=False):
    x = np.asarray(x, dtype=np.float32)
    Wq = np.asarray(Wq, dtype=np.float32)
    Wk = np.asarray(Wk, dtype=np.float32)
    Wv = np.asarray(Wv, dtype=np.float32)
    nc = build_nc()
    in_maps = _host_prep(x, Wq, Wk, Wv)
    res = run_bass_kernel_spmd(nc, in_maps, core_ids=list(range(NCORES)),
                               trace=trace)
    out = np.empty((B, S, D), dtype=np.float32)
    for core in range(NCORES):
        b, h = divmod(core, 2)
        c_lo, c_hi = h, 3 - h
        o = res.results[core]["out"]
        out[b, c_lo * 512:(c_lo + 1) * 512] = o[:512]
        out[b, c_hi * 512:(c_hi + 1) * 512] = o[512:]
    return out, res


def kernel(x, Wq, Wk, Wv):
    out, _ = run(x, Wq, Wk, Wv)
    return out


if __name__ == "__main__":
    build_nc()
    print("build + compile OK")


# revision 30
# speedup vs baseline: 1.2357x; 1.0149x over previous
"""Causal attention kernel for Trainium2, 8 NeuronCores — depth-split fp8.

Problem: x[4,2048,2048] @ Wq/Wk/Wv[2048,2048] -> causal softmax attention.

Sharding (as baseline): 2 cores per batch; each core owns 1024 query rows as
global 512-row chunks {0,3} (even cores) / {1,2} (odd cores). Pairwise
AllGather assembles full K^T / V per batch. Gathered key-block positions are
[chunk0, chunk3, chunk1, chunk2] on every core; query slot 0 (c_lo) attends
positions {0-3, 8-11}, slot 1 (c_hi) all 16; causality via mask tensors.

Depth-split fp8: rows in chunks 2,3 (slot 1, >=1025 visible keys) have
diffuse softmax, so fp8-e4m3 noise (~4%/tensor) attenuates to <1% there:
  - slot-1 Q/K/V projections in fp8 DoubleRow (2 fp8/PE cell, 256-deep
    contraction per pass), weights host-scaled x64, staged back at 1/64.
  - slot-1 scores fp8 DoubleRow; exp shifted by -2 so unnormalized fp8
    probs stay < 240 (TRN e4m3 max); shift cancels in normalization.
  - slot-1 AV in fp8 DoubleRow (adjacent key-block pairs; odd-length
    causal runs rounded up — the padded block's probs are mask-zeroed).
  - slot-0 (chunks 0,1) stays bf16 end-to-end: shallow rows concentrate
    softmax mass and pass v through nearly verbatim (sim: bf16 4.9e-3,
    full-fp8 4.5e-2, this split 1.33e-2 vs the 2e-2 gate).
c_lo K/V are computed in bf16 and dual-staged (bf16 + fp8 copy); c_hi rows
never need bf16 x.

Scheduling: staging stores ride the gpsimd queue (engine-local DIRECT2D);
the 9 collectives are emitted so nothing tensor-critical queues behind
their peer-sync waits — in particular Q staging writes PSUM->SBUF straight
into the phase-2 q tiles (no DRAM round-trip, no gpsimd dependency).
First x/W tiles are split small so the first matmul waits on ~0.75MB.
"""

import math

import numpy as np
import ml_dtypes

import concourse.bass as bass
import concourse.mybir as mybir
import concourse.tile as tile
from concourse import bacc
from concourse.bass import ds, ts
from concourse.bass_utils import run_bass_kernel_spmd

B, S, D = 4, 2048, 2048
P = 128
DC = D // P          # 16 contraction chunks
SB = S // P          # 16 key blocks
QROWS = 1024         # query rows per core
NCORES = 8
INV_SQRT_D = 1.0 / math.sqrt(D)
WS = 64.0            # host scale on fp8 weights (keeps W8 in e4m3 normal range)
ESHIFT = -2.0        # exp shift: unnormalized fp8 probs < 240

# gathered key-block position -> true 512-chunk (pair-rank order, all cores)
POS2TRUE = [0, 3, 1, 2]
SLOT0_POS = [0, 1, 2, 3, 8, 9, 10, 11]   # slot-0's (bf16) key positions
S0IDX = {pos: j for j, pos in enumerate(SLOT0_POS)}
PAIRS = [[0, 1], [2, 3], [4, 5], [6, 7]]

F32 = mybir.dt.float32
BF16 = mybir.dt.bfloat16
F8 = mybir.dt.float8e4
DR = mybir.MatmulPerfMode.DoubleRow
Exp = mybir.ActivationFunctionType.Exp
Copy = mybir.ActivationFunctionType.Copy

_CACHED_NC = None


def build_nc():
    global _CACHED_NC
    if _CACHED_NC is not None:
        return _CACHED_NC
    nc = bacc.Bacc(trn_type="TRN2", target_bir_lowering=False, debug=False,
                   num_devices=NCORES)

    # x^T shipped pre-tiled as [P, DC, 512] so SBUF loads are single DMAs
    xtb_d = nc.dram_tensor("xtb", [P, DC, 512], BF16, kind="ExternalInput")
    xt8_d = nc.dram_tensor("xt8", [P, DC, 512], F8, kind="ExternalInput")
    wqb_d = nc.dram_tensor("wqb", [DC, P, DC, P], BF16, kind="ExternalInput")
    wq8_d = nc.dram_tensor("wq8", [DC, P, DC, P], F8, kind="ExternalInput")
    wkb_d = nc.dram_tensor("wkb", [DC, P, DC, P], BF16, kind="ExternalInput")
    wk8_d = nc.dram_tensor("wk8", [DC, P, DC, P], F8, kind="ExternalInput")
    wvb_d = nc.dram_tensor("wvb", [4, 2, P, 8, 512], BF16, kind="ExternalInput")
    wv8_d = nc.dram_tensor("wv8", [4, 2, P, 8, 512], F8, kind="ExternalInput")
    mkb_d = nc.dram_tensor("maskb", [P, 8, 512], BF16, kind="ExternalInput")
    mk8_d = nc.dram_tensor("mask8", [P, 16, 512], F8, kind="ExternalInput")
    out_d = nc.dram_tensor("out", [QROWS, D], F32, kind="ExternalOutput")

    with tile.TileContext(nc) as tc:
        with (
            tc.tile_pool(name="dram", bufs=1, space="DRAM") as dpool,
            tc.tile_pool(name="ps", bufs=8, space="PSUM") as ps_all,
            tc.tile_pool(name="qsb", bufs=1) as qsb_pool,
        ):
            # phase-2 q tiles, written directly by Q staging (PSUM->SBUF)
            qtb = qsb_pool.tile([P, DC, 512], BF16, tag="qtb")
            qt8 = qsb_pool.tile([P, DC, 512], F8, tag="qt8")

            # [P, 8, 4, P] m-half layout: a whole [P,512] staging tile
            # stores with ONE dma; each half gathers as soon as its 8 m's
            # are staged, spreading collective traffic into the K window
            kTb_own = [dpool.tile([P, 8, 4, P], BF16, tag=f"kTbo{h}",
                                  name=f"kTbo{h}") for h in range(2)]
            kT8_own = [[dpool.tile([P, 8, 4, P], F8, tag=f"kT8o{s}{h}",
                                   name=f"kT8o{s}{h}") for h in range(2)]
                       for s in range(2)]
            kgb = [dpool.tile([2, P, 8, 4, P], BF16, tag=f"kgb{h}",
                              name=f"kgb{h}") for h in range(2)]
            kg8 = [[dpool.tile([2, P, 8, 4, P], F8, tag=f"kg8{s}{h}",
                               name=f"kg8{s}{h}") for h in range(2)]
                   for s in range(2)]
            vvb_own = [dpool.tile([2, P, D], BF16, tag=f"vvbo{g}",
                                  name=f"vvbo{g}") for g in range(2)]
            vv8_own = [dpool.tile([2, P, D], F8, tag=f"vv8o{g}",
                                  name=f"vv8o{g}") for g in range(4)]
            vgb = [dpool.tile([2, 2, P, D], BF16, tag=f"vgb{g}",
                              name=f"vgb{g}") for g in range(2)]
            vg8 = [dpool.tile([2, 2, P, D], F8, tag=f"vg8{g}",
                              name=f"vg8{g}") for g in range(4)]

            # ---------------- phase 1: projections ----------------
            with (
                tc.tile_pool(name="xt", bufs=1) as xt_pool,
                tc.tile_pool(name="wbf", bufs=20) as wbf_pool,
                tc.tile_pool(name="w8", bufs=16) as w8_pool,
                tc.tile_pool(name="wv", bufs=5) as wv_pool,
                tc.tile_pool(name="wv8", bufs=4) as wv8_pool,
                tc.tile_pool(name="st", bufs=8) as st_pool,
                tc.tile_pool(name="st8", bufs=8) as st8_pool,
                tc.tile_pool(name="sv", bufs=5) as sv_pool,
                tc.tile_pool(name="sv8", bufs=6) as sv8_pool,
            ):
                # bf16 W tiles split in half so the first matmul's deps
                # are small (dep tracking is tile-granular)
                def load_wb(dram, m, name):
                    lo = wbf_pool.tile([P, 8, P], BF16, tag="w",
                                       name=f"{name}l")
                    nc.sync.dma_start(lo[:], dram.ap()[m][:, :8, :])
                    hi = wbf_pool.tile([P, 8, P], BF16, tag="w",
                                       name=f"{name}h")
                    nc.sync.dma_start(hi[:], dram.ap()[m][:, 8:, :])
                    return lo, hi

                def wb_ap(wpair, dc):
                    return wpair[dc // 8][:, dc % 8, :]

                def load_w8(dram, m, name):
                    wt = w8_pool.tile([P, DC, P], F8, tag="w", name=name)
                    nc.sync.dma_start(wt[:], dram.ap()[m])
                    return wt

                wkb = [load_wb(wkb_d, 0, "wkb0")]
                # x^T c_lo split into 4 tiles so the first matmuls' deps
                # are 0.5MB each
                xtbs = []
                for c in range(4):
                    t = xt_pool.tile([P, 4, 512], BF16, tag=f"xtb{c}",
                                     name=f"xtb{c}")
                    nc.sync.dma_start(t[:], xtb_d.ap()[:, ds(4 * c, 4), :])
                    xtbs.append(t)

                def xtb_ap(dc):
                    return xtbs[dc // 4][:, dc % 4, :]

                wkb += [load_wb(wkb_d, m, f"wkb{m}") for m in range(1, 8)]
                xt8 = xt_pool.tile([P, DC, 512], F8, tag="xt8")
                nc.sync.dma_start(xt8[:, :8, :], xt8_d.ap()[:, :8, :])
                nc.sync.dma_start(xt8[:, 8:, :], xt8_d.ap()[:, 8:, :])
                wkb += [load_wb(wkb_d, m, f"wkb{m}") for m in range(8, DC)]
                wk8 = [load_w8(wk8_d, m, f"wk8{m}") for m in range(DC)]

                # --- K c_lo rows: bf16, dual-staged (bf16 + fp8)
                for m in range(DC):
                    ps = ps_all.tile([P, 512], F32, tag="ps")
                    for dc in range(DC):
                        nc.tensor.matmul(
                            ps[:], lhsT=wb_ap(wkb[m], dc), rhs=xtb_ap(dc),
                            start=(dc == 0), stop=(dc == DC - 1),
                        )
                    stb = st_pool.tile([P, 512], BF16, tag="st")
                    nc.scalar.copy(stb[:], ps[:])
                    st8 = st8_pool.tile([P, 512], F8, tag="st8")
                    nc.vector.tensor_copy(st8[:], ps[:])
                    nc.scalar.dma_start(kTb_own[m // 8][:, m % 8, :, :],
                                        stb[:])
                    nc.scalar.dma_start(kT8_own[0][m // 8][:, m % 8, :, :],
                                        st8[:])
                    if m % 8 == 7:
                        h = m // 8
                        nc.gpsimd.collective_compute(
                            "AllGather", mybir.AluOpType.bypass,
                            replica_groups=PAIRS,
                            ins=[kTb_own[h].opt()], outs=[kgb[h].opt()])
                        nc.gpsimd.collective_compute(
                            "AllGather", mybir.AluOpType.bypass,
                            replica_groups=PAIRS,
                            ins=[kT8_own[0][h].opt()],
                            outs=[kg8[0][h].opt()])

                # --- K c_hi rows: fp8 DoubleRow
                for m in range(DC):
                    ps = ps_all.tile([P, 512], F32, tag="ps")
                    for t in range(8):
                        nc.tensor.matmul(
                            ps[:], lhsT=wk8[m][:, ds(2 * t, 2), :],
                            rhs=xt8[:, ds(2 * t, 2), :],
                            start=(t == 0), stop=(t == 7), perf_mode=DR,
                        )
                    st8 = st8_pool.tile([P, 512], F8, tag="st8")
                    nc.scalar.activation(st8[:], ps[:], Copy, scale=1.0 / WS)
                    nc.scalar.dma_start(kT8_own[1][m // 8][:, m % 8, :, :],
                                        st8[:])
                    if m % 8 == 7:
                        h = m // 8
                        nc.gpsimd.collective_compute(
                            "AllGather", mybir.AluOpType.bypass,
                            replica_groups=PAIRS,
                            ins=[kT8_own[1][h].opt()],
                            outs=[kg8[1][h].opt()])

                # --- V: n-outer so only 2 wv tile pairs are resident
                def load_wv(pool, dram, n, hb, dt, name):
                    wvt = pool.tile([P, 8, 512], dt, tag="wv", name=name)
                    nc.sync.dma_start(wvt[:], dram.ap()[n, hb])
                    return wvt

                wvb_t = {}
                wv8_t = {}
                for n in range(2):
                    for hb in range(2):
                        wvb_t[n, hb] = load_wv(wv_pool, wvb_d, n, hb, BF16,
                                               f"wvb{n}{hb}")
                        wv8_t[n, hb] = load_wv(wv8_pool, wv8_d, n, hb, F8,
                                               f"wv8{n}{hb}")

                # Q weight loads: emitted here so they stream during K/V
                wqb = [load_wb(wqb_d, m, f"wqb{m}") for m in range(DC)]
                wq8 = [load_w8(wq8_d, m, f"wq8{m}") for m in range(DC)]

                for n in range(4):
                    if n + 2 < 4:
                        for hb in range(2):
                            wvb_t[n + 2, hb] = load_wv(
                                wv_pool, wvb_d, n + 2, hb, BF16,
                                f"wvb{n + 2}{hb}")
                            wv8_t[n + 2, hb] = load_wv(
                                wv8_pool, wv8_d, n + 2, hb, F8,
                                f"wv8{n + 2}{hb}")
                    # c_lo rows: bf16, dual-staged
                    for s in range(4):
                        ps = ps_all.tile([P, 512], F32, tag="ps")
                        for dc in range(DC):
                            w = wvb_t[n, dc // 8]
                            nc.tensor.matmul(
                                ps[:], lhsT=xtb_ap(dc)[:, ts(s, P)],
                                rhs=w[:, dc % 8, :],
                                start=(dc == 0), stop=(dc == DC - 1),
                            )
                        svb = sv_pool.tile([P, 512], BF16, tag="sv")
                        nc.vector.tensor_copy(svb[:], ps[:])
                        sv8 = sv8_pool.tile([P, 512], F8, tag="sv8")
                        nc.scalar.copy(sv8[:], ps[:])
                        nc.scalar.dma_start(
                            vvb_own[s // 2][s % 2, :, ts(n, 512)], svb[:])
                        nc.scalar.dma_start(
                            vv8_own[s // 2][s % 2, :, ts(n, 512)], sv8[:])
                    # c_hi rows: fp8 DoubleRow
                    for s in range(4):
                        ps = ps_all.tile([P, 512], F32, tag="ps")
                        for t in range(8):
                            w = wv8_t[n, t // 4]
                            nc.tensor.matmul(
                                ps[:], lhsT=xt8[:, ds(2 * t, 2), ts(s, P)],
                                rhs=w[:, ds(2 * (t % 4), 2), :],
                                start=(t == 0), stop=(t == 7), perf_mode=DR,
                            )
                        sv8 = sv8_pool.tile([P, 512], F8, tag="sv8")
                        nc.scalar.activation(sv8[:], ps[:], Copy,
                                             scale=1.0 / WS)
                        nc.scalar.dma_start(
                            vv8_own[2 + s // 2][s % 2, :, ts(n, 512)], sv8[:])

                # --- Q: c_lo bf16 / c_hi fp8 DR, staged straight into the
                # phase-2 SBUF q tiles (no DRAM round-trip, no gpsimd dep:
                # the V AllGathers below can't stall Q)
                for m in range(DC):
                    ps = ps_all.tile([P, 512], F32, tag="ps")
                    for dc in range(DC):
                        nc.tensor.matmul(
                            ps[:], lhsT=wb_ap(wqb[m], dc), rhs=xtb_ap(dc),
                            start=(dc == 0), stop=(dc == DC - 1),
                        )
                    nc.scalar.copy(qtb[:, m, :], ps[:])
                for m in range(DC):
                    ps = ps_all.tile([P, 512], F32, tag="ps")
                    for t in range(8):
                        nc.tensor.matmul(
                            ps[:], lhsT=wq8[m][:, ds(2 * t, 2), :],
                            rhs=xt8[:, ds(2 * t, 2), :],
                            start=(t == 0), stop=(t == 7), perf_mode=DR,
                        )
                    nc.scalar.activation(qt8[:, m, :], ps[:], Copy,
                                         scale=1.0 / WS)

                for g in range(2):
                    nc.gpsimd.collective_compute(
                        "AllGather", mybir.AluOpType.bypass,
                        replica_groups=PAIRS,
                        ins=[vvb_own[g].opt()], outs=[vgb[g].opt()])
                for g in range(4):
                    nc.gpsimd.collective_compute(
                        "AllGather", mybir.AluOpType.bypass,
                        replica_groups=PAIRS,
                        ins=[vv8_own[g].opt()], outs=[vg8[g].opt()])

            # ---------------- phase 2: attention ----------------
            with (
                tc.tile_pool(name="pt", bufs=1) as pt_pool,
                tc.tile_pool(name="mk", bufs=1) as mk_pool,
                tc.tile_pool(name="vb", bufs=1) as vb_pool,
                tc.tile_pool(name="ktb", bufs=8) as ktb_pool,
                tc.tile_pool(name="kt8", bufs=16) as kt8_pool,
                tc.tile_pool(name="one", bufs=1) as one_pool,
                tc.tile_pool(name="sc", bufs=4) as sc_pool,
                tc.tile_pool(name="ob", bufs=4) as ob_pool,
            ):
                mkb = mk_pool.tile([P, 8, 512], BF16, tag="mkb")
                nc.sync.dma_start(mkb[:, :4, :], mkb_d.ap()[:, :4, :])
                nc.sync.dma_start(mkb[:, 4:, :], mkb_d.ap()[:, 4:, :])
                mk8 = mk_pool.tile([P, 16, 512], F8, tag="mk8")
                nc.sync.dma_start(mk8[:, :8, :], mk8_d.ap()[:, :8, :])
                nc.sync.dma_start(mk8[:, 8:, :], mk8_d.ap()[:, 8:, :])
                onesb = one_pool.tile([P, 1], BF16, tag="onesb")
                nc.vector.memset(onesb[:], 1.0)
                ones8 = one_pool.tile([P, 2, 1], F8, tag="ones8")
                nc.vector.memset(ones8[:], 1.0)
                ebias = one_pool.tile([P, 1], F32, tag="ebias")
                nc.vector.memset(ebias[:], ESHIFT)

                ptb = pt_pool.tile([P, 8, 512], BF16, tag="ptb")
                pt8 = pt_pool.tile([P, 16, 512], F8, tag="pt8")

                # all kt tiles preloaded (fully resident, single-dma each),
                # then the V big tiles — kt loads must dispatch first so
                # score matmuls aren't gated by vbig's gather waits
                def kt8_src(pos, h):
                    lo = pos in (0, 1, 2, 3, 8, 9, 10, 11)
                    return kg8[0 if lo else 1][h][pos // 8][:, :, pos % 4, :]

                kt8_ts = []
                ktb_ts = {}
                for pos in range(SB):
                    kt8_t = kt8_pool.tile([P, DC, P], F8, tag="kt8",
                                          name=f"kt8_{pos}")
                    nc.sync.dma_start(kt8_t[:, :8, :], kt8_src(pos, 0))
                    nc.sync.dma_start(kt8_t[:, 8:, :], kt8_src(pos, 1))
                    kt8_ts.append(kt8_t)
                    if pos in S0IDX:
                        ktb_t = ktb_pool.tile([P, DC, P], BF16, tag="ktb",
                                              name=f"ktb_{pos}")
                        nc.sync.dma_start(
                            ktb_t[:, :8, :],
                            kgb[0][pos // 8][:, :, pos % 4, :])
                        nc.sync.dma_start(
                            ktb_t[:, 8:, :],
                            kgb[1][pos // 8][:, :, pos % 4, :])
                        ktb_ts[pos] = ktb_t

                vbb = vb_pool.tile([P, 8, D], BF16, tag="vbb")
                for jj, pos in enumerate(SLOT0_POS):
                    nc.sync.dma_start(
                        vbb[:, jj, :],
                        vgb[(pos % 8) // 2][pos // 8, pos % 2])
                v8b = vb_pool.tile([P, 16, D], F8, tag="v8b")
                for pos in range(SB):
                    nc.sync.dma_start(
                        v8b[:, pos, :],
                        vg8[(pos % 8) // 2][pos // 8, pos % 2])

                for pos in range(SB):
                    kt8_t = kt8_ts[pos]
                    ps = ps_all.tile([P, 512], F32, tag="ps")
                    for t in range(8):
                        nc.tensor.matmul(
                            ps[:], lhsT=kt8_t[:, ds(2 * t, 2), :],
                            rhs=qt8[:, ds(2 * t, 2), :],
                            start=(t == 0), stop=(t == 7), perf_mode=DR,
                        )
                    nc.scalar.activation(pt8[:, pos, :], ps[:], Exp,
                                         scale=INV_SQRT_D, bias=ebias[:])
                    if pos in S0IDX:
                        ktb_t = ktb_ts[pos]
                        ps2 = ps_all.tile([P, 512], F32, tag="ps")
                        for dc in range(DC):
                            nc.tensor.matmul(
                                ps2[:], lhsT=ktb_t[:, dc, :],
                                rhs=qtb[:, dc, :],
                                start=(dc == 0), stop=(dc == DC - 1),
                            )
                        nc.scalar.activation(ptb[:, S0IDX[pos], :], ps2[:],
                                             Exp, scale=INV_SQRT_D)

                for j in range(8):
                    nc.vector.tensor_mul(ptb[:, j, :], ptb[:, j, :],
                                         mkb[:, j, :])
                for pos in range(SB):
                    nc.vector.tensor_mul(pt8[:, pos, :], pt8[:, pos, :],
                                         mk8[:, pos, :])

                # --- slot 0 (bf16): rowsum, AV, normalize, out rows 0..511
                for qs in range(4):
                    kpos = [0, 1, 2, 3] + list(range(8, 9 + qs))
                    idx = [S0IDX[p] for p in kpos]
                    plt = ps_all.tile([P, 512], F32, tag="ps", name="pl")
                    pl = plt[:, :1]
                    for i, j in enumerate(idx):
                        nc.tensor.matmul(
                            pl[:], lhsT=ptb[:, j, ts(qs, P)], rhs=onesb[:],
                            start=(i == 0), stop=(i == len(idx) - 1),
                        )
                    rl = sc_pool.tile([P, 1], F32, tag="rl")
                    nc.vector.reciprocal(rl[:], pl[:])
                    for n in range(4):
                        pav = ps_all.tile([P, 512], F32, tag="ps", name="pav")
                        for i, j in enumerate(idx):
                            nc.tensor.matmul(
                                pav[:], lhsT=ptb[:, j, ts(qs, P)],
                                rhs=vbb[:, j, ts(n, 512)],
                                start=(i == 0), stop=(i == len(idx) - 1),
                            )
                        ob = ob_pool.tile([P, 512], F32, tag="ob")
                        if n % 2 == 0:
                            nc.vector.tensor_scalar_mul(ob[:], pav[:], rl[:])
                        else:
                            nc.scalar.activation(ob[:], pav[:], Copy,
                                                 scale=rl[:])
                        nc.sync.dma_start(
                            out_d.ap()[ds(qs * P, P), ts(n, 512)], ob[:])

                # --- slot 1 (fp8 DR): rows 512..1023
                for qs in range(4):
                    run1 = 6 if qs < 2 else 8   # [0..4+qs] rounded to even
                    pstarts = list(range(0, run1, 2)) + [8, 10, 12, 14]
                    plt = ps_all.tile([P, 512], F32, tag="ps", name="pl8")
                    pl = plt[:, :1]
                    for i, p0 in enumerate(pstarts):
                        nc.tensor.matmul(
                            pl[:], lhsT=pt8[:, ds(p0, 2), ts(qs, P)],
                            rhs=ones8[:],
                            start=(i == 0), stop=(i == len(pstarts) - 1),
                            perf_mode=DR,
                        )
                    rl = sc_pool.tile([P, 1], F32, tag="rl")
                    nc.vector.reciprocal(rl[:], pl[:])
                    for n in range(4):
                        pav = ps_all.tile([P, 512], F32, tag="ps", name="pav8")
                        for i, p0 in enumerate(pstarts):
                            nc.tensor.matmul(
                                pav[:], lhsT=pt8[:, ds(p0, 2), ts(qs, P)],
                                rhs=v8b[:, ds(p0, 2), ts(n, 512)],
                                start=(i == 0), stop=(i == len(pstarts) - 1),
                                perf_mode=DR,
                            )
                        ob = ob_pool.tile([P, 512], F32, tag="ob")
                        if n % 2 == 0:
                            nc.vector.tensor_scalar_mul(ob[:], pav[:], rl[:])
                        else:
                            nc.scalar.activation(ob[:], pav[:], Copy,
                                                 scale=rl[:])
                        nc.sync.dma_start(
                            out_d.ap()[ds(512 + qs * P, P), ts(n, 512)], ob[:])

    nc.compile()
    _CACHED_NC = nc
    return nc


def _host_prep(x, Wq, Wk, Wv):
    """Build per-core input maps (host-side layout prep)."""
    f8 = ml_dtypes.float8_e4m3
    bf = ml_dtypes.bfloat16

    def wqk_layout(W, dt, scale=1.0):
        return np.ascontiguousarray(
            (W * scale).reshape(DC, P, DC, P).transpose(2, 1, 0, 3)).astype(dt)

    def wv_layout(W, dt, scale=1.0):
        return np.ascontiguousarray(
            (W * scale).reshape(2, 8, P, 4, 512).transpose(3, 0, 2, 1, 4)
        ).astype(dt)

    wqb_h = wqk_layout(Wq, bf)
    wq8_h = wqk_layout(Wq, f8, WS)
    wkb_h = wqk_layout(Wk, bf)
    wk8_h = wqk_layout(Wk, f8, WS)
    wvb_h = wv_layout(Wv, bf)
    wv8_h = wv_layout(Wv, f8, WS)

    k_in_block = np.arange(P, dtype=np.int64)[:, None]           # [P, 1]
    q_in_chunk = np.arange(512, dtype=np.int64)[None, :]         # [1, 512]

    def build_masks(h):
        c_lo, c_hi = h, 3 - h
        mb = np.zeros((P, 8, 512), dtype=bf)
        for j, pos in enumerate(SLOT0_POS):
            tkb = POS2TRUE[pos // 4] * 4 + pos % 4
            mb[:, j, :] = (tkb * P + k_in_block) <= (c_lo * 512 + q_in_chunk)
        m8 = np.zeros((P, 16, 512), dtype=f8)
        for pos in range(SB):
            tkb = POS2TRUE[pos // 4] * 4 + pos % 4
            m8[:, pos, :] = (tkb * P + k_in_block) <= (c_hi * 512 + q_in_chunk)
        return mb, m8

    mask_h = [build_masks(0), build_masks(1)]

    in_maps = []
    for core in range(NCORES):
        b, h = divmod(core, 2)
        c_lo, c_hi = h, 3 - h
        xt = x[b].T                                               # [D, S] view
        xtb_h = np.ascontiguousarray(
            xt[:, c_lo * 512:(c_lo + 1) * 512].reshape(DC, P, 512)
            .transpose(1, 0, 2)).astype(bf)
        xt8_h = np.ascontiguousarray(
            xt[:, c_hi * 512:(c_hi + 1) * 512].reshape(DC, P, 512)
            .transpose(1, 0, 2)).astype(f8)
        in_maps.append({
            "xtb": xtb_h,
            "xt8": xt8_h,
            "wqb": wqb_h, "wq8": wq8_h, "wkb": wkb_h, "wk8": wk8_h,
            "wvb": wvb_h, "wv8": wv8_h,
            "maskb": mask_h[h][0], "mask8": mask_h[h][1],
        })
    return in_maps


def run(x, Wq, Wk, Wv, trace=False):
    x = np.asarray(x, dtype=np.float32)
    Wq = np.asarray(Wq, dtype=np.float32)
    Wk = np.asarray(Wk, dtype=np.float32)
    Wv = np.asarray(Wv, dtype=np.float32)
    nc = build_nc()
    in_maps = _host_prep(x, Wq, Wk, Wv)
    res = run_bass_kernel_spmd(nc, in_maps, core_ids=list(range(NCORES)),
                               trace=trace)
    out = np.empty((B, S, D), dtype=np.float32)
    for core in range(NCORES):
        b, h = divmod(core, 2)
        c_lo, c_hi = h, 3 - h
        o = res.results[core]["out"]
        out[b, c_lo * 512:(c_lo + 1) * 512] = o[:512]
        out[b, c_hi * 512:(c_hi + 1) * 512] = o[512:]
    return out, res


def kernel(x, Wq, Wk, Wv):
    out, _ = run(x, Wq, Wk, Wv)
    return out


if __name__ == "__main__":
    build_nc()
    print("build + compile OK")


# revision 31
# speedup vs baseline: 1.2401x; 1.0036x over previous
"""Causal attention kernel for Trainium2, 8 NeuronCores — depth-split fp8.

Problem: x[4,2048,2048] @ Wq/Wk/Wv[2048,2048] -> causal softmax attention.

Sharding (as baseline): 2 cores per batch; each core owns 1024 query rows as
global 512-row chunks {0,3} (even cores) / {1,2} (odd cores). Pairwise
AllGather assembles full K^T / V per batch. Gathered key-block positions are
[chunk0, chunk3, chunk1, chunk2] on every core; query slot 0 (c_lo) attends
positions {0-3, 8-11}, slot 1 (c_hi) all 16; causality via mask tensors.

Depth-split fp8: rows in chunks 2,3 (slot 1, >=1025 visible keys) have
diffuse softmax, so fp8-e4m3 noise (~4%/tensor) attenuates to <1% there:
  - slot-1 Q/K/V projections in fp8 DoubleRow (2 fp8/PE cell, 256-deep
    contraction per pass), weights host-scaled x64, staged back at 1/64.
  - slot-1 scores fp8 DoubleRow; exp shifted by -2 so unnormalized fp8
    probs stay < 240 (TRN e4m3 max); shift cancels in normalization.
  - slot-1 AV in fp8 DoubleRow (adjacent key-block pairs; odd-length
    causal runs rounded up — the padded block's probs are mask-zeroed).
  - slot-0 (chunks 0,1) stays bf16 end-to-end: shallow rows concentrate
    softmax mass and pass v through nearly verbatim (sim: bf16 4.9e-3,
    full-fp8 4.5e-2, this split 1.33e-2 vs the 2e-2 gate).
c_lo K/V are computed in bf16 and dual-staged (bf16 + fp8 copy); c_hi rows
never need bf16 x.

Scheduling: staging stores ride the gpsimd queue (engine-local DIRECT2D);
the 9 collectives are emitted so nothing tensor-critical queues behind
their peer-sync waits — in particular Q staging writes PSUM->SBUF straight
into the phase-2 q tiles (no DRAM round-trip, no gpsimd dependency).
First x/W tiles are split small so the first matmul waits on ~0.75MB.
"""

import math

import numpy as np
import ml_dtypes

import concourse.bass as bass
import concourse.mybir as mybir
import concourse.tile as tile
from concourse import bacc
from concourse.bass import ds, ts
from concourse.bass_utils import run_bass_kernel_spmd

B, S, D = 4, 2048, 2048
P = 128
DC = D // P          # 16 contraction chunks
SB = S // P          # 16 key blocks
QROWS = 1024         # query rows per core
NCORES = 8
INV_SQRT_D = 1.0 / math.sqrt(D)
WS = 64.0            # host scale on fp8 weights (keeps W8 in e4m3 normal range)
ESHIFT = -2.0        # exp shift: unnormalized fp8 probs < 240

# gathered key-block position -> true 512-chunk (pair-rank order, all cores)
POS2TRUE = [0, 3, 1, 2]
SLOT0_POS = [0, 1, 2, 3, 8, 9, 10, 11]   # slot-0's (bf16) key positions
S0IDX = {pos: j for j, pos in enumerate(SLOT0_POS)}
PAIRS = [[0, 1], [2, 3], [4, 5], [6, 7]]

F32 = mybir.dt.float32
BF16 = mybir.dt.bfloat16
F8 = mybir.dt.float8e4
DR = mybir.MatmulPerfMode.DoubleRow
Exp = mybir.ActivationFunctionType.Exp
Copy = mybir.ActivationFunctionType.Copy

_CACHED_NC = None


def build_nc():
    global _CACHED_NC
    if _CACHED_NC is not None:
        return _CACHED_NC
    nc = bacc.Bacc(trn_type="TRN2", target_bir_lowering=False, debug=False,
                   num_devices=NCORES)

    # x^T shipped pre-tiled as [P, DC, 512] so SBUF loads are single DMAs
    xtb_d = nc.dram_tensor("xtb", [P, DC, 512], BF16, kind="ExternalInput")
    xt8_d = nc.dram_tensor("xt8", [P, DC, 512], F8, kind="ExternalInput")
    wqb_d = nc.dram_tensor("wqb", [DC, P, DC, P], BF16, kind="ExternalInput")
    wq8_d = nc.dram_tensor("wq8", [DC, P, DC, P], F8, kind="ExternalInput")
    wkb_d = nc.dram_tensor("wkb", [DC, P, DC, P], BF16, kind="ExternalInput")
    wk8_d = nc.dram_tensor("wk8", [DC, P, DC, P], F8, kind="ExternalInput")
    wvb_d = nc.dram_tensor("wvb", [4, 2, P, 8, 512], BF16, kind="ExternalInput")
    wv8_d = nc.dram_tensor("wv8", [4, 2, P, 8, 512], F8, kind="ExternalInput")
    mkb_d = nc.dram_tensor("maskb", [P, 8, 512], BF16, kind="ExternalInput")
    mk8_d = nc.dram_tensor("mask8", [P, 16, 512], F8, kind="ExternalInput")
    out_d = nc.dram_tensor("out", [QROWS, D], F32, kind="ExternalOutput")

    with tile.TileContext(nc) as tc:
        with (
            tc.tile_pool(name="dram", bufs=1, space="DRAM") as dpool,
            tc.tile_pool(name="ps", bufs=8, space="PSUM") as ps_all,
            tc.tile_pool(name="qsb", bufs=1) as qsb_pool,
        ):
            # phase-2 q tiles, written directly by Q staging (PSUM->SBUF)
            qtb = qsb_pool.tile([P, DC, 512], BF16, tag="qtb")
            qt8 = qsb_pool.tile([P, DC, 512], F8, tag="qt8")

            # [P, 8, 4, P] m-half layout: a whole [P,512] staging tile
            # stores with ONE dma; each half gathers as soon as its 8 m's
            # are staged, spreading collective traffic into the K window
            kTb_own = [dpool.tile([P, 8, 4, P], BF16, tag=f"kTbo{h}",
                                  name=f"kTbo{h}") for h in range(2)]
            kT8_own = [dpool.tile([P, 8, 4, P], F8, tag=f"kT8o{h}",
                                  name=f"kT8o{h}") for h in range(2)]
            kgb = [dpool.tile([2, P, 8, 4, P], BF16, tag=f"kgb{h}",
                              name=f"kgb{h}") for h in range(2)]
            kg8 = [dpool.tile([2, P, 8, 4, P], F8, tag=f"kg8{h}",
                              name=f"kg8{h}") for h in range(2)]
            vvb_own = [dpool.tile([2, P, D], BF16, tag=f"vvbo{g}",
                                  name=f"vvbo{g}") for g in range(2)]
            vv8_own = [dpool.tile([2, P, D], F8, tag=f"vv8o{g}",
                                  name=f"vv8o{g}") for g in range(2)]
            vgb = [dpool.tile([2, 2, P, D], BF16, tag=f"vgb{g}",
                              name=f"vgb{g}") for g in range(2)]
            vg8 = [dpool.tile([2, 2, P, D], F8, tag=f"vg8{g}",
                              name=f"vg8{g}") for g in range(2)]

            # ---------------- phase 1: projections ----------------
            with (
                tc.tile_pool(name="xt", bufs=1) as xt_pool,
                tc.tile_pool(name="wbf", bufs=20) as wbf_pool,
                tc.tile_pool(name="w8", bufs=16) as w8_pool,
                tc.tile_pool(name="wv", bufs=5) as wv_pool,
                tc.tile_pool(name="wv8", bufs=4) as wv8_pool,
                tc.tile_pool(name="st", bufs=8) as st_pool,
                tc.tile_pool(name="st8", bufs=8) as st8_pool,
                tc.tile_pool(name="sv", bufs=5) as sv_pool,
                tc.tile_pool(name="sv8", bufs=6) as sv8_pool,
            ):
                # bf16 W tiles split in half so the first matmul's deps
                # are small (dep tracking is tile-granular)
                def load_wb(dram, m, name):
                    lo = wbf_pool.tile([P, 8, P], BF16, tag="w",
                                       name=f"{name}l")
                    nc.sync.dma_start(lo[:], dram.ap()[m][:, :8, :])
                    hi = wbf_pool.tile([P, 8, P], BF16, tag="w",
                                       name=f"{name}h")
                    nc.sync.dma_start(hi[:], dram.ap()[m][:, 8:, :])
                    return lo, hi

                def wb_ap(wpair, dc):
                    return wpair[dc // 8][:, dc % 8, :]

                def load_w8(dram, m, name):
                    wt = w8_pool.tile([P, DC, P], F8, tag="w", name=name)
                    nc.sync.dma_start(wt[:], dram.ap()[m])
                    return wt

                wkb = [load_wb(wkb_d, 0, "wkb0")]
                # x^T c_lo split into 4 tiles so the first matmuls' deps
                # are 0.5MB each
                xtbs = []
                for c in range(8):
                    t = xt_pool.tile([P, 2, 512], BF16, tag=f"xtb{c}",
                                     name=f"xtb{c}")
                    nc.sync.dma_start(t[:], xtb_d.ap()[:, ds(2 * c, 2), :])
                    xtbs.append(t)

                def xtb_ap(dc):
                    return xtbs[dc // 2][:, dc % 2, :]

                wkb += [load_wb(wkb_d, m, f"wkb{m}") for m in range(1, 8)]
                xt8 = xt_pool.tile([P, DC, 512], F8, tag="xt8")
                nc.sync.dma_start(xt8[:, :8, :], xt8_d.ap()[:, :8, :])
                nc.sync.dma_start(xt8[:, 8:, :], xt8_d.ap()[:, 8:, :])
                wkb += [load_wb(wkb_d, m, f"wkb{m}") for m in range(8, DC)]
                wk8 = [load_w8(wk8_d, m, f"wk8{m}") for m in range(DC)]

                # --- K c_lo rows: bf16, dual-staged (bf16 + fp8)
                for m in range(DC):
                    ps = ps_all.tile([P, 512], F32, tag="ps")
                    for dc in range(DC):
                        nc.tensor.matmul(
                            ps[:], lhsT=wb_ap(wkb[m], dc), rhs=xtb_ap(dc),
                            start=(dc == 0), stop=(dc == DC - 1),
                        )
                    stb = st_pool.tile([P, 512], BF16, tag="st")
                    nc.scalar.copy(stb[:], ps[:])
                    nc.scalar.dma_start(kTb_own[m // 8][:, m % 8, :, :],
                                        stb[:])
                    if m % 8 == 7:
                        h = m // 8
                        nc.gpsimd.collective_compute(
                            "AllGather", mybir.AluOpType.bypass,
                            replica_groups=PAIRS,
                            ins=[kTb_own[h].opt()], outs=[kgb[h].opt()])

                # --- K c_hi rows: fp8 DoubleRow
                for m in range(DC):
                    ps = ps_all.tile([P, 512], F32, tag="ps")
                    for t in range(8):
                        nc.tensor.matmul(
                            ps[:], lhsT=wk8[m][:, ds(2 * t, 2), :],
                            rhs=xt8[:, ds(2 * t, 2), :],
                            start=(t == 0), stop=(t == 7), perf_mode=DR,
                        )
                    st8 = st8_pool.tile([P, 512], F8, tag="st8")
                    nc.scalar.activation(st8[:], ps[:], Copy, scale=1.0 / WS)
                    nc.scalar.dma_start(kT8_own[m // 8][:, m % 8, :, :],
                                        st8[:])
                    if m % 8 == 7:
                        h = m // 8
                        nc.gpsimd.collective_compute(
                            "AllGather", mybir.AluOpType.bypass,
                            replica_groups=PAIRS,
                            ins=[kT8_own[h].opt()],
                            outs=[kg8[h].opt()])

                # --- V: n-outer so only 2 wv tile pairs are resident
                def load_wv(pool, dram, n, hb, dt, name):
                    wvt = pool.tile([P, 8, 512], dt, tag="wv", name=name)
                    nc.sync.dma_start(wvt[:], dram.ap()[n, hb])
                    return wvt

                wvb_t = {}
                wv8_t = {}
                for n in range(2):
                    for hb in range(2):
                        wvb_t[n, hb] = load_wv(wv_pool, wvb_d, n, hb, BF16,
                                               f"wvb{n}{hb}")
                        wv8_t[n, hb] = load_wv(wv8_pool, wv8_d, n, hb, F8,
                                               f"wv8{n}{hb}")

                # Q weight loads: emitted here so they stream during K/V
                wqb = [load_wb(wqb_d, m, f"wqb{m}") for m in range(DC)]
                wq8 = [load_w8(wq8_d, m, f"wq8{m}") for m in range(DC)]

                for n in range(4):
                    if n + 2 < 4:
                        for hb in range(2):
                            wvb_t[n + 2, hb] = load_wv(
                                wv_pool, wvb_d, n + 2, hb, BF16,
                                f"wvb{n + 2}{hb}")
                            wv8_t[n + 2, hb] = load_wv(
                                wv8_pool, wv8_d, n + 2, hb, F8,
                                f"wv8{n + 2}{hb}")
                    # c_lo rows: bf16, dual-staged
                    for s in range(4):
                        ps = ps_all.tile([P, 512], F32, tag="ps")
                        for dc in range(DC):
                            w = wvb_t[n, dc // 8]
                            nc.tensor.matmul(
                                ps[:], lhsT=xtb_ap(dc)[:, ts(s, P)],
                                rhs=w[:, dc % 8, :],
                                start=(dc == 0), stop=(dc == DC - 1),
                            )
                        svb = sv_pool.tile([P, 512], BF16, tag="sv")
                        nc.vector.tensor_copy(svb[:], ps[:])
                        nc.scalar.dma_start(
                            vvb_own[s // 2][s % 2, :, ts(n, 512)], svb[:])
                    # c_hi rows: fp8 DoubleRow
                    for s in range(4):
                        ps = ps_all.tile([P, 512], F32, tag="ps")
                        for t in range(8):
                            w = wv8_t[n, t // 4]
                            nc.tensor.matmul(
                                ps[:], lhsT=xt8[:, ds(2 * t, 2), ts(s, P)],
                                rhs=w[:, ds(2 * (t % 4), 2), :],
                                start=(t == 0), stop=(t == 7), perf_mode=DR,
                            )
                        sv8 = sv8_pool.tile([P, 512], F8, tag="sv8")
                        nc.scalar.activation(sv8[:], ps[:], Copy,
                                             scale=1.0 / WS)
                        nc.scalar.dma_start(
                            vv8_own[s // 2][s % 2, :, ts(n, 512)], sv8[:])

                # --- Q: c_lo bf16 / c_hi fp8 DR, staged straight into the
                # phase-2 SBUF q tiles (no DRAM round-trip, no gpsimd dep:
                # the V AllGathers below can't stall Q)
                for m in range(DC):
                    ps = ps_all.tile([P, 512], F32, tag="ps")
                    for dc in range(DC):
                        nc.tensor.matmul(
                            ps[:], lhsT=wb_ap(wqb[m], dc), rhs=xtb_ap(dc),
                            start=(dc == 0), stop=(dc == DC - 1),
                        )
                    nc.scalar.copy(qtb[:, m, :], ps[:])
                for m in range(DC):
                    ps = ps_all.tile([P, 512], F32, tag="ps")
                    for t in range(8):
                        nc.tensor.matmul(
                            ps[:], lhsT=wq8[m][:, ds(2 * t, 2), :],
                            rhs=xt8[:, ds(2 * t, 2), :],
                            start=(t == 0), stop=(t == 7), perf_mode=DR,
                        )
                    nc.scalar.activation(qt8[:, m, :], ps[:], Copy,
                                         scale=1.0 / WS)

                for g in range(2):
                    nc.gpsimd.collective_compute(
                        "AllGather", mybir.AluOpType.bypass,
                        replica_groups=PAIRS,
                        ins=[vvb_own[g].opt()], outs=[vgb[g].opt()])
                for g in range(2):
                    nc.gpsimd.collective_compute(
                        "AllGather", mybir.AluOpType.bypass,
                        replica_groups=PAIRS,
                        ins=[vv8_own[g].opt()], outs=[vg8[g].opt()])

            # ---------------- phase 2: attention ----------------
            with (
                tc.tile_pool(name="pt", bufs=1) as pt_pool,
                tc.tile_pool(name="mk", bufs=1) as mk_pool,
                tc.tile_pool(name="vb", bufs=1) as vb_pool,
                tc.tile_pool(name="ktb", bufs=8) as ktb_pool,
                tc.tile_pool(name="kt8", bufs=16) as kt8_pool,
                tc.tile_pool(name="one", bufs=1) as one_pool,
                tc.tile_pool(name="sc", bufs=4) as sc_pool,
                tc.tile_pool(name="ob", bufs=4) as ob_pool,
            ):
                mkb = mk_pool.tile([P, 8, 512], BF16, tag="mkb")
                nc.sync.dma_start(mkb[:, :4, :], mkb_d.ap()[:, :4, :])
                nc.sync.dma_start(mkb[:, 4:, :], mkb_d.ap()[:, 4:, :])
                mk8 = mk_pool.tile([P, 16, 512], F8, tag="mk8")
                nc.sync.dma_start(mk8[:, :8, :], mk8_d.ap()[:, :8, :])
                nc.sync.dma_start(mk8[:, 8:, :], mk8_d.ap()[:, 8:, :])
                onesb = one_pool.tile([P, 1], BF16, tag="onesb")
                nc.vector.memset(onesb[:], 1.0)
                ones8 = one_pool.tile([P, 2, 1], F8, tag="ones8")
                nc.vector.memset(ones8[:], 1.0)
                ebias = one_pool.tile([P, 1], F32, tag="ebias")
                nc.vector.memset(ebias[:], ESHIFT)

                ptb = pt_pool.tile([P, 8, 512], BF16, tag="ptb")
                pt8 = pt_pool.tile([P, 16, 512], F8, tag="pt8")

                # all kt tiles preloaded (fully resident, single-dma each),
                # then the V big tiles — kt loads must dispatch first so
                # score matmuls aren't gated by vbig's gather waits
                kt8_ts = []
                ktb_ts = {}
                for pos in range(SB):
                    kt8_t = kt8_pool.tile([P, DC, P], F8, tag="kt8",
                                          name=f"kt8_{pos}")
                    if pos in S0IDX:
                        # c_lo keys: load bf16 (slot-0 needs it anyway) and
                        # cast to fp8 on the idle vector engine — no fp8
                        # gather for these at all
                        ktb_t = ktb_pool.tile([P, DC, P], BF16, tag="ktb",
                                              name=f"ktb_{pos}")
                        nc.sync.dma_start(
                            ktb_t[:, :8, :],
                            kgb[0][pos // 8][:, :, pos % 4, :])
                        nc.sync.dma_start(
                            ktb_t[:, 8:, :],
                            kgb[1][pos // 8][:, :, pos % 4, :])
                        ktb_ts[pos] = ktb_t
                        nc.vector.tensor_copy(kt8_t[:], ktb_t[:])
                    else:
                        nc.sync.dma_start(
                            kt8_t[:, :8, :],
                            kg8[0][(pos // 8)][:, :, pos % 4, :])
                        nc.sync.dma_start(
                            kt8_t[:, 8:, :],
                            kg8[1][(pos // 8)][:, :, pos % 4, :])
                    kt8_ts.append(kt8_t)

                vbb = vb_pool.tile([P, 8, D], BF16, tag="vbb")
                for jj, pos in enumerate(SLOT0_POS):
                    nc.sync.dma_start(
                        vbb[:, jj, :],
                        vgb[(pos % 8) // 2][pos // 8, pos % 2])
                v8b = vb_pool.tile([P, 16, D], F8, tag="v8b")
                for pos in range(SB):
                    if pos in S0IDX:
                        # cast c_lo V from the bf16 copy (gpsimd queue is
                        # free once the collectives have launched)
                        nc.gpsimd.tensor_copy(v8b[:, pos, :],
                                              vbb[:, S0IDX[pos], :])
                    else:
                        nc.sync.dma_start(
                            v8b[:, pos, :],
                            vg8[(pos % 8) // 2 - 2][pos // 8, pos % 2])

                for pos in range(SB):
                    kt8_t = kt8_ts[pos]
                    ps = ps_all.tile([P, 512], F32, tag="ps")
                    for t in range(8):
                        nc.tensor.matmul(
                            ps[:], lhsT=kt8_t[:, ds(2 * t, 2), :],
                            rhs=qt8[:, ds(2 * t, 2), :],
                            start=(t == 0), stop=(t == 7), perf_mode=DR,
                        )
                    nc.scalar.activation(pt8[:, pos, :], ps[:], Exp,
                                         scale=INV_SQRT_D, bias=ebias[:])
                    if pos in S0IDX:
                        ktb_t = ktb_ts[pos]
                        ps2 = ps_all.tile([P, 512], F32, tag="ps")
                        for dc in range(DC):
                            nc.tensor.matmul(
                                ps2[:], lhsT=ktb_t[:, dc, :],
                                rhs=qtb[:, dc, :],
                                start=(dc == 0), stop=(dc == DC - 1),
                            )
                        nc.scalar.activation(ptb[:, S0IDX[pos], :], ps2[:],
                                             Exp, scale=INV_SQRT_D)

                for j in range(8):
                    nc.vector.tensor_mul(ptb[:, j, :], ptb[:, j, :],
                                         mkb[:, j, :])
                for pos in range(SB):
                    nc.vector.tensor_mul(pt8[:, pos, :], pt8[:, pos, :],
                                         mk8[:, pos, :])

                # --- slot 0 (bf16): rowsum, AV, normalize, out rows 0..511
                for qs in range(4):
                    kpos = [0, 1, 2, 3] + list(range(8, 9 + qs))
                    idx = [S0IDX[p] for p in kpos]
                    plt = ps_all.tile([P, 512], F32, tag="ps", name="pl")
                    pl = plt[:, :1]
                    for i, j in enumerate(idx):
                        nc.tensor.matmul(
                            pl[:], lhsT=ptb[:, j, ts(qs, P)], rhs=onesb[:],
                            start=(i == 0), stop=(i == len(idx) - 1),
                        )
                    rl = sc_pool.tile([P, 1], F32, tag="rl")
                    nc.vector.reciprocal(rl[:], pl[:])
                    for n in range(4):
                        pav = ps_all.tile([P, 512], F32, tag="ps", name="pav")
                        for i, j in enumerate(idx):
                            nc.tensor.matmul(
                                pav[:], lhsT=ptb[:, j, ts(qs, P)],
                                rhs=vbb[:, j, ts(n, 512)],
                                start=(i == 0), stop=(i == len(idx) - 1),
                            )
                        ob = ob_pool.tile([P, 512], F32, tag="ob")
                        if n % 2 == 0:
                            nc.vector.tensor_scalar_mul(ob[:], pav[:], rl[:])
                        else:
                            nc.scalar.activation(ob[:], pav[:], Copy,
                                                 scale=rl[:])
                        nc.sync.dma_start(
                            out_d.ap()[ds(qs * P, P), ts(n, 512)], ob[:])

                # --- slot 1 (fp8 DR): rows 512..1023
                for qs in range(4):
                    run1 = 6 if qs < 2 else 8   # [0..4+qs] rounded to even
                    pstarts = list(range(0, run1, 2)) + [8, 10, 12, 14]
                    plt = ps_all.tile([P, 512], F32, tag="ps", name="pl8")
                    pl = plt[:, :1]
                    for i, p0 in enumerate(pstarts):
                        nc.tensor.matmul(
                            pl[:], lhsT=pt8[:, ds(p0, 2), ts(qs, P)],
                            rhs=ones8[:],
                            start=(i == 0), stop=(i == len(pstarts) - 1),
                            perf_mode=DR,
                        )
                    rl = sc_pool.tile([P, 1], F32, tag="rl")
                    nc.vector.reciprocal(rl[:], pl[:])
                    for n in range(4):
                        pav = ps_all.tile([P, 512], F32, tag="ps", name="pav8")
                        for i, p0 in enumerate(pstarts):
                            nc.tensor.matmul(
                                pav[:], lhsT=pt8[:, ds(p0, 2), ts(qs, P)],
                                rhs=v8b[:, ds(p0, 2), ts(n, 512)],
                                start=(i == 0), stop=(i == len(pstarts) - 1),
                                perf_mode=DR,
                            )
                        ob = ob_pool.tile([P, 512], F32, tag="ob")
                        if n % 2 == 0:
                            nc.vector.tensor_scalar_mul(ob[:], pav[:], rl[:])
                        else:
                            nc.scalar.activation(ob[:], pav[:], Copy,
                                                 scale=rl[:])
                        nc.sync.dma_start(
                            out_d.ap()[ds(512 + qs * P, P), ts(n, 512)], ob[:])

    nc.compile()
    _CACHED_NC = nc
    return nc


def _host_prep(x, Wq, Wk, Wv):
    """Build per-core input maps (host-side layout prep)."""
    f8 = ml_dtypes.float8_e4m3
    bf = ml_dtypes.bfloat16

    def wqk_layout(W, dt, scale=1.0):
        return np.ascontiguousarray(
            (W * scale).reshape(DC, P, DC, P).transpose(2, 1, 0, 3)).astype(dt)

    def wv_layout(W, dt, scale=1.0):
        return np.ascontiguousarray(
            (W * scale).reshape(2, 8, P, 4, 512).transpose(3, 0, 2, 1, 4)
        ).astype(dt)

    wqb_h = wqk_layout(Wq, bf)
    wq8_h = wqk_layout(Wq, f8, WS)
    wkb_h = wqk_layout(Wk, bf)
    wk8_h = wqk_layout(Wk, f8, WS)
    wvb_h = wv_layout(Wv, bf)
    wv8_h = wv_layout(Wv, f8, WS)

    k_in_block = np.arange(P, dtype=np.int64)[:, None]           # [P, 1]
    q_in_chunk = np.arange(512, dtype=np.int64)[None, :]         # [1, 512]

    def build_masks(h):
        c_lo, c_hi = h, 3 - h
        mb = np.zeros((P, 8, 512), dtype=bf)
        for j, pos in enumerate(SLOT0_POS):
            tkb = POS2TRUE[pos // 4] * 4 + pos % 4
            mb[:, j, :] = (tkb * P + k_in_block) <= (c_lo * 512 + q_in_chunk)
        m8 = np.zeros((P, 16, 512), dtype=f8)
        for pos in range(SB):
            tkb = POS2TRUE[pos // 4] * 4 + pos % 4
            m8[:, pos, :] = (tkb * P + k_in_block) <= (c_hi * 512 + q_in_chunk)
        return mb, m8

    mask_h = [build_masks(0), build_masks(1)]

    in_maps = []
    for core in range(NCORES):
        b, h = divmod(core, 2)
        c_lo, c_hi = h, 3 - h
        xt = x[b].T                                               # [D, S] view
        xtb_h = np.ascontiguousarray(
            xt[:, c_lo * 512:(c_lo + 1) * 512].reshape(DC, P, 512)
            .transpose(1, 0, 2)).astype(bf)
        xt8_h = np.ascontiguousarray(
            xt[:, c_hi * 512:(c_hi + 1) * 512].reshape(DC, P, 512)
            .transpose(1, 0, 2)).astype(f8)
        in_maps.append({
            "xtb": xtb_h,
            "xt8": xt8_h,
            "wqb": wqb_h, "wq8": wq8_h, "wkb": wkb_h, "wk8": wk8_h,
            "wvb": wvb_h, "wv8": wv8_h,
            "maskb": mask_h[h][0], "mask8": mask_h[h][1],
        })
    return in_maps


def run(x, Wq, Wk, Wv, trace=False):
    x = np.asarray(x, dtype=np.float32)
    Wq = np.asarray(Wq, dtype=np.float32)
    Wk = np.asarray(Wk, dtype=np.float32)
    Wv = np.asarray(Wv, dtype=np.float32)
    nc = build_nc()
    in_maps = _host_prep(x, Wq, Wk, Wv)
    res = run_bass_kernel_spmd(nc, in_maps, core_ids=list(range(NCORES)),
                               trace=trace)
    out = np.empty((B, S, D), dtype=np.float32)
    for core in range(NCORES):
        b, h = divmod(core, 2)
        c_lo, c_hi = h, 3 - h
        o = res.results[core]["out"]
        out[b, c_lo * 512:(c_lo + 1) * 512] = o[:512]
        out[b, c_hi * 512:(c_hi + 1) * 512] = o[512:]
    return out, res


def kernel(x, Wq, Wk, Wv):
    out, _ = run(x, Wq, Wk, Wv)
    return out


if __name__ == "__main__":
    build_nc()
    print("build + compile OK")


# revision 32
# speedup vs baseline: 1.2926x; 1.0424x over previous
"""Causal attention kernel for Trainium2, 8 NeuronCores — depth-split fp8.

Problem: x[4,2048,2048] @ Wq/Wk/Wv[2048,2048] -> causal softmax attention.

Sharding (as baseline): 2 cores per batch; each core owns 1024 query rows as
global 512-row chunks {0,3} (even cores) / {1,2} (odd cores). Pairwise
AllGather assembles full K^T / V per batch. Gathered key-block positions are
[chunk0, chunk3, chunk1, chunk2] on every core; query slot 0 (c_lo) attends
positions {0-3, 8-11}, slot 1 (c_hi) all 16; causality via mask tensors.

Depth-split fp8: rows in chunks 2,3 (slot 1, >=1025 visible keys) have
diffuse softmax, so fp8-e4m3 noise (~4%/tensor) attenuates to <1% there:
  - slot-1 Q/K/V projections in fp8 DoubleRow (2 fp8/PE cell, 256-deep
    contraction per pass), weights host-scaled x64, staged back at 1/64.
  - slot-1 scores fp8 DoubleRow; exp shifted by -2 so unnormalized fp8
    probs stay < 240 (TRN e4m3 max); shift cancels in normalization.
  - slot-1 AV in fp8 DoubleRow (adjacent key-block pairs; odd-length
    causal runs rounded up — the padded block's probs are mask-zeroed).
  - slot-0 (chunks 0,1) stays bf16 end-to-end: shallow rows concentrate
    softmax mass and pass v through nearly verbatim (sim: bf16 4.9e-3,
    full-fp8 4.5e-2, this split 1.33e-2 vs the 2e-2 gate).
c_lo K/V are computed in bf16 and dual-staged (bf16 + fp8 copy); c_hi rows
never need bf16 x.

Scheduling: staging stores ride the gpsimd queue (engine-local DIRECT2D);
the 9 collectives are emitted so nothing tensor-critical queues behind
their peer-sync waits — in particular Q staging writes PSUM->SBUF straight
into the phase-2 q tiles (no DRAM round-trip, no gpsimd dependency).
First x/W tiles are split small so the first matmul waits on ~0.75MB.
"""

import math

import numpy as np
import ml_dtypes

import concourse.bass as bass
import concourse.mybir as mybir
import concourse.tile as tile
from concourse import bacc
from concourse.bass import ds, ts
from concourse.bass_utils import run_bass_kernel_spmd

B, S, D = 4, 2048, 2048
P = 128
DC = D // P          # 16 contraction chunks
SB = S // P          # 16 key blocks
QROWS = 1024         # query rows per core
NCORES = 8
INV_SQRT_D = 1.0 / math.sqrt(D)
WS = 64.0            # host scale on fp8 weights (keeps W8 in e4m3 normal range)
ESHIFT = -2.0        # exp shift: unnormalized fp8 probs < 240

# gathered key-block position -> true 512-chunk (pair-rank order, all cores)
POS2TRUE = [0, 3, 1, 2]
SLOT0_POS = [0, 1, 2, 3, 8, 9, 10, 11]   # slot-0's (bf16) key positions
S0IDX = {pos: j for j, pos in enumerate(SLOT0_POS)}
PAIRS = [[0, 1], [2, 3], [4, 5], [6, 7]]

F32 = mybir.dt.float32
BF16 = mybir.dt.bfloat16
F8 = mybir.dt.float8e4
DR = mybir.MatmulPerfMode.DoubleRow
Exp = mybir.ActivationFunctionType.Exp
Copy = mybir.ActivationFunctionType.Copy

_CACHED_NC = None


def build_nc():
    global _CACHED_NC
    if _CACHED_NC is not None:
        return _CACHED_NC
    nc = bacc.Bacc(trn_type="TRN2", target_bir_lowering=False, debug=False,
                   num_devices=NCORES)

    # x^T shipped pre-tiled as [P, DC, 512] so SBUF loads are single DMAs
    xtb_d = nc.dram_tensor("xtb", [P, DC, 512], BF16, kind="ExternalInput")
    xt8_d = nc.dram_tensor("xt8", [P, DC, 512], F8, kind="ExternalInput")
    wqb_d = nc.dram_tensor("wqb", [DC, P, DC, P], BF16, kind="ExternalInput")
    wq8_d = nc.dram_tensor("wq8", [DC, P, DC, P], F8, kind="ExternalInput")
    wkb_d = nc.dram_tensor("wkb", [DC, P, DC, P], BF16, kind="ExternalInput")
    wk8_d = nc.dram_tensor("wk8", [DC, P, DC, P], F8, kind="ExternalInput")
    wvb_d = nc.dram_tensor("wvb", [4, 2, P, 8, 512], BF16, kind="ExternalInput")
    wv8_d = nc.dram_tensor("wv8", [4, 2, P, 8, 512], F8, kind="ExternalInput")
    mkb_d = nc.dram_tensor("maskb", [P, 8, 512], BF16, kind="ExternalInput")
    mk8_d = nc.dram_tensor("mask8", [P, 16, 512], F8, kind="ExternalInput")
    out_d = nc.dram_tensor("out", [QROWS, D], F32, kind="ExternalOutput")

    with tile.TileContext(nc) as tc:
        with (
            tc.tile_pool(name="dram", bufs=1, space="DRAM") as dpool,
            tc.tile_pool(name="ps", bufs=8, space="PSUM") as ps_all,
            tc.tile_pool(name="qsb", bufs=1) as qsb_pool,
        ):
            # phase-2 q tiles, written directly by Q staging (PSUM->SBUF)
            qtb = qsb_pool.tile([P, DC, 512], BF16, tag="qtb")
            qt8 = qsb_pool.tile([P, DC, 512], F8, tag="qt8")

            # [P, 8, 4, P] m-half layout: a whole [P,512] staging tile
            # stores with ONE dma; each half gathers as soon as its 8 m's
            # are staged, spreading collective traffic into the K window
            kTb_own = [dpool.tile([P, 8, 4, P], BF16, tag=f"kTbo{h}",
                                  name=f"kTbo{h}") for h in range(2)]
            kT8_own = [dpool.tile([P, 8, 4, P], F8, tag=f"kT8o{h}",
                                  name=f"kT8o{h}") for h in range(2)]
            kgb = [dpool.tile([2, P, 8, 4, P], BF16, tag=f"kgb{h}",
                              name=f"kgb{h}") for h in range(2)]
            kg8 = [dpool.tile([2, P, 8, 4, P], F8, tag=f"kg8{h}",
                              name=f"kg8{h}") for h in range(2)]
            vvb_own = [dpool.tile([2, P, D], BF16, tag=f"vvbo{g}",
                                  name=f"vvbo{g}") for g in range(2)]
            vv8_own = [dpool.tile([2, P, D], F8, tag=f"vv8o{g}",
                                  name=f"vv8o{g}") for g in range(2)]
            vgb = [dpool.tile([2, 2, P, D], BF16, tag=f"vgb{g}",
                              name=f"vgb{g}") for g in range(2)]
            vg8 = [dpool.tile([2, 2, P, D], F8, tag=f"vg8{g}",
                              name=f"vg8{g}") for g in range(2)]

            # ---------------- phase 1: projections ----------------
            with (
                tc.tile_pool(name="xt", bufs=1) as xt_pool,
                tc.tile_pool(name="wbf", bufs=20) as wbf_pool,
                tc.tile_pool(name="w8", bufs=16) as w8_pool,
                tc.tile_pool(name="wv", bufs=5) as wv_pool,
                tc.tile_pool(name="wv8", bufs=4) as wv8_pool,
                tc.tile_pool(name="st", bufs=8) as st_pool,
                tc.tile_pool(name="st8", bufs=8) as st8_pool,
                tc.tile_pool(name="sv", bufs=5) as sv_pool,
                tc.tile_pool(name="sv8", bufs=6) as sv8_pool,
            ):
                # bf16 W tiles split in half so the first matmul's deps
                # are small (dep tracking is tile-granular)
                def load_wb(dram, m, name):
                    lo = wbf_pool.tile([P, 8, P], BF16, tag="w",
                                       name=f"{name}l")
                    nc.sync.dma_start(lo[:], dram.ap()[m][:, :8, :])
                    hi = wbf_pool.tile([P, 8, P], BF16, tag="w",
                                       name=f"{name}h")
                    nc.sync.dma_start(hi[:], dram.ap()[m][:, 8:, :])
                    return lo, hi

                def wb_ap(wpair, dc):
                    return wpair[dc // 8][:, dc % 8, :]

                def load_w8(dram, m, name):
                    wt = w8_pool.tile([P, DC, P], F8, tag="w", name=name)
                    nc.sync.dma_start(wt[:], dram.ap()[m])
                    return wt

                wkb = [load_wb(wkb_d, 0, "wkb0")]
                # x^T c_lo split into 4 tiles so the first matmuls' deps
                # are 0.5MB each
                xtbs = []
                for c in range(8):
                    t = xt_pool.tile([P, 2, 512], BF16, tag=f"xtb{c}",
                                     name=f"xtb{c}")
                    nc.sync.dma_start(t[:], xtb_d.ap()[:, ds(2 * c, 2), :])
                    xtbs.append(t)

                def xtb_ap(dc):
                    return xtbs[dc // 2][:, dc % 2, :]

                wkb += [load_wb(wkb_d, m, f"wkb{m}") for m in range(1, 8)]
                xt8 = xt_pool.tile([P, DC, 512], F8, tag="xt8")
                nc.sync.dma_start(xt8[:, :8, :], xt8_d.ap()[:, :8, :])
                nc.sync.dma_start(xt8[:, 8:, :], xt8_d.ap()[:, 8:, :])
                wkb += [load_wb(wkb_d, m, f"wkb{m}") for m in range(8, DC)]
                wk8 = [load_w8(wk8_d, m, f"wk8{m}") for m in range(DC)]

                # --- K c_lo rows: bf16, dual-staged (bf16 + fp8)
                for m in range(DC):
                    ps = ps_all.tile([P, 512], F32, tag="ps")
                    for dc in range(DC):
                        nc.tensor.matmul(
                            ps[:], lhsT=wb_ap(wkb[m], dc), rhs=xtb_ap(dc),
                            start=(dc == 0), stop=(dc == DC - 1),
                        )
                    stb = st_pool.tile([P, 512], BF16, tag="st")
                    nc.scalar.copy(stb[:], ps[:])
                    nc.scalar.dma_start(kTb_own[m // 8][:, m % 8, :, :],
                                        stb[:])
                    if m % 8 == 7:
                        h = m // 8
                        nc.gpsimd.collective_compute(
                            "AllGather", mybir.AluOpType.bypass,
                            replica_groups=PAIRS,
                            ins=[kTb_own[h].opt()], outs=[kgb[h].opt()])

                # --- K c_hi rows: fp8 DoubleRow
                for m in range(DC):
                    ps = ps_all.tile([P, 512], F32, tag="ps")
                    for t in range(8):
                        nc.tensor.matmul(
                            ps[:], lhsT=wk8[m][:, ds(2 * t, 2), :],
                            rhs=xt8[:, ds(2 * t, 2), :],
                            start=(t == 0), stop=(t == 7), perf_mode=DR,
                        )
                    st8 = st8_pool.tile([P, 512], F8, tag="st8")
                    nc.scalar.activation(st8[:], ps[:], Copy, scale=1.0 / WS)
                    nc.scalar.dma_start(kT8_own[m // 8][:, m % 8, :, :],
                                        st8[:])
                    if m % 8 == 7:
                        h = m // 8
                        nc.gpsimd.collective_compute(
                            "AllGather", mybir.AluOpType.bypass,
                            replica_groups=PAIRS,
                            ins=[kT8_own[h].opt()],
                            outs=[kg8[h].opt()])

                # --- V: n-outer so only 2 wv tile pairs are resident
                def load_wv(pool, dram, n, hb, dt, name):
                    wvt = pool.tile([P, 8, 512], dt, tag="wv", name=name)
                    nc.sync.dma_start(wvt[:], dram.ap()[n, hb])
                    return wvt

                wvb_t = {}
                wv8_t = {}
                for n in range(2):
                    for hb in range(2):
                        wvb_t[n, hb] = load_wv(wv_pool, wvb_d, n, hb, BF16,
                                               f"wvb{n}{hb}")
                        wv8_t[n, hb] = load_wv(wv8_pool, wv8_d, n, hb, F8,
                                               f"wv8{n}{hb}")

                # Q weight loads: emitted here so they stream during K/V
                wqb = [load_wb(wqb_d, m, f"wqb{m}") for m in range(DC)]
                wq8 = [load_w8(wq8_d, m, f"wq8{m}") for m in range(DC)]

                for n in range(4):
                    if n + 2 < 4:
                        for hb in range(2):
                            wvb_t[n + 2, hb] = load_wv(
                                wv_pool, wvb_d, n + 2, hb, BF16,
                                f"wvb{n + 2}{hb}")
                            wv8_t[n + 2, hb] = load_wv(
                                wv8_pool, wv8_d, n + 2, hb, F8,
                                f"wv8{n + 2}{hb}")
                    # c_lo rows: bf16, dual-staged
                    for s in range(4):
                        ps = ps_all.tile([P, 512], F32, tag="ps")
                        for dc in range(DC):
                            w = wvb_t[n, dc // 8]
                            nc.tensor.matmul(
                                ps[:], lhsT=xtb_ap(dc)[:, ts(s, P)],
                                rhs=w[:, dc % 8, :],
                                start=(dc == 0), stop=(dc == DC - 1),
                            )
                        svb = sv_pool.tile([P, 512], BF16, tag="sv")
                        nc.vector.tensor_copy(svb[:], ps[:])
                        nc.scalar.dma_start(
                            vvb_own[s // 2][s % 2, :, ts(n, 512)], svb[:])
                    # c_hi rows: fp8 DoubleRow
                    for s in range(4):
                        ps = ps_all.tile([P, 512], F32, tag="ps")
                        for t in range(8):
                            w = wv8_t[n, t // 4]
                            nc.tensor.matmul(
                                ps[:], lhsT=xt8[:, ds(2 * t, 2), ts(s, P)],
                                rhs=w[:, ds(2 * (t % 4), 2), :],
                                start=(t == 0), stop=(t == 7), perf_mode=DR,
                            )
                        sv8 = sv8_pool.tile([P, 512], F8, tag="sv8")
                        nc.scalar.activation(sv8[:], ps[:], Copy,
                                             scale=1.0 / WS)
                        nc.scalar.dma_start(
                            vv8_own[s // 2][s % 2, :, ts(n, 512)], sv8[:])

                # --- Q: c_lo bf16 / c_hi fp8 DR, staged straight into the
                # phase-2 SBUF q tiles (no DRAM round-trip, no gpsimd dep:
                # the V AllGathers below can't stall Q)
                for m in range(DC):
                    ps = ps_all.tile([P, 512], F32, tag="ps")
                    for dc in range(DC):
                        nc.tensor.matmul(
                            ps[:], lhsT=wb_ap(wqb[m], dc), rhs=xtb_ap(dc),
                            start=(dc == 0), stop=(dc == DC - 1),
                        )
                    nc.scalar.copy(qtb[:, m, :], ps[:])
                for m in range(DC):
                    ps = ps_all.tile([P, 512], F32, tag="ps")
                    for t in range(8):
                        nc.tensor.matmul(
                            ps[:], lhsT=wq8[m][:, ds(2 * t, 2), :],
                            rhs=xt8[:, ds(2 * t, 2), :],
                            start=(t == 0), stop=(t == 7), perf_mode=DR,
                        )
                    nc.scalar.activation(qt8[:, m, :], ps[:], Copy,
                                         scale=1.0 / WS)

                for g in range(2):
                    nc.gpsimd.collective_compute(
                        "AllGather", mybir.AluOpType.bypass,
                        replica_groups=PAIRS,
                        ins=[vvb_own[g].opt()], outs=[vgb[g].opt()])
                for g in range(2):
                    nc.gpsimd.collective_compute(
                        "AllGather", mybir.AluOpType.bypass,
                        replica_groups=PAIRS,
                        ins=[vv8_own[g].opt()], outs=[vg8[g].opt()])

            # ---------------- phase 2: attention ----------------
            with (
                tc.tile_pool(name="pt", bufs=1) as pt_pool,
                tc.tile_pool(name="mk", bufs=1) as mk_pool,
                tc.tile_pool(name="vb", bufs=1) as vb_pool,
                tc.tile_pool(name="ktb", bufs=8) as ktb_pool,
                tc.tile_pool(name="kt8", bufs=16) as kt8_pool,
                tc.tile_pool(name="one", bufs=1) as one_pool,
                tc.tile_pool(name="sc", bufs=4) as sc_pool,
                tc.tile_pool(name="ob", bufs=4) as ob_pool,
            ):
                mkb = mk_pool.tile([P, 8, 512], BF16, tag="mkb")
                nc.sync.dma_start(mkb[:, :4, :], mkb_d.ap()[:, :4, :])
                nc.sync.dma_start(mkb[:, 4:, :], mkb_d.ap()[:, 4:, :])
                mk8 = mk_pool.tile([P, 16, 512], F8, tag="mk8")
                nc.sync.dma_start(mk8[:, :8, :], mk8_d.ap()[:, :8, :])
                nc.sync.dma_start(mk8[:, 8:, :], mk8_d.ap()[:, 8:, :])
                onesb = one_pool.tile([P, 1], BF16, tag="onesb")
                nc.vector.memset(onesb[:], 1.0)
                ones8 = one_pool.tile([P, 2, 1], F8, tag="ones8")
                nc.vector.memset(ones8[:], 1.0)
                ebias = one_pool.tile([P, 1], F32, tag="ebias")
                nc.vector.memset(ebias[:], ESHIFT)

                ptb = pt_pool.tile([P, 8, 512], BF16, tag="ptb")
                pt8 = pt_pool.tile([P, 16, 512], F8, tag="pt8")

                # all kt tiles preloaded (fully resident, single-dma each),
                # then the V big tiles — kt loads must dispatch first so
                # score matmuls aren't gated by vbig's gather waits
                kt8_ts = []
                ktb_ts = {}
                for pos in range(SB):
                    kt8_t = kt8_pool.tile([P, DC, P], F8, tag="kt8",
                                          name=f"kt8_{pos}")
                    if pos in S0IDX:
                        # c_lo keys: load bf16 (slot-0 needs it anyway) and
                        # cast to fp8 on the idle vector engine — no fp8
                        # gather for these at all
                        ktb_t = ktb_pool.tile([P, DC, P], BF16, tag="ktb",
                                              name=f"ktb_{pos}")
                        nc.sync.dma_start(
                            ktb_t[:, :8, :],
                            kgb[0][pos // 8][:, :, pos % 4, :])
                        nc.sync.dma_start(
                            ktb_t[:, 8:, :],
                            kgb[1][pos // 8][:, :, pos % 4, :])
                        ktb_ts[pos] = ktb_t
                        nc.vector.tensor_copy(kt8_t[:], ktb_t[:])
                    else:
                        nc.sync.dma_start(
                            kt8_t[:, :8, :],
                            kg8[0][(pos // 8)][:, :, pos % 4, :])
                        nc.sync.dma_start(
                            kt8_t[:, 8:, :],
                            kg8[1][(pos // 8)][:, :, pos % 4, :])
                    kt8_ts.append(kt8_t)

                vbb = vb_pool.tile([P, 8, D], BF16, tag="vbb")
                for jj, pos in enumerate(SLOT0_POS):
                    nc.sync.dma_start(
                        vbb[:, jj, :],
                        vgb[(pos % 8) // 2][pos // 8, pos % 2])
                v8b = vb_pool.tile([P, 16, D], F8, tag="v8b")
                for pos in range(SB):
                    if pos in S0IDX:
                        # cast c_lo V from the bf16 copy (scalar engine:
                        # RNE conversion; gpsimd truncates)
                        nc.scalar.copy(v8b[:, pos, :],
                                       vbb[:, S0IDX[pos], :])
                    else:
                        nc.sync.dma_start(
                            v8b[:, pos, :],
                            vg8[(pos % 8) // 2 - 2][pos // 8, pos % 2])

                for pos in range(SB):
                    kt8_t = kt8_ts[pos]
                    ps = ps_all.tile([P, 512], F32, tag="ps")
                    for t in range(8):
                        nc.tensor.matmul(
                            ps[:], lhsT=kt8_t[:, ds(2 * t, 2), :],
                            rhs=qt8[:, ds(2 * t, 2), :],
                            start=(t == 0), stop=(t == 7), perf_mode=DR,
                        )
                    nc.scalar.activation(pt8[:, pos, :], ps[:], Exp,
                                         scale=INV_SQRT_D, bias=ebias[:])
                    if pos in S0IDX:
                        ktb_t = ktb_ts[pos]
                        ps2 = ps_all.tile([P, 512], F32, tag="ps")
                        for dc in range(DC):
                            nc.tensor.matmul(
                                ps2[:], lhsT=ktb_t[:, dc, :],
                                rhs=qtb[:, dc, :],
                                start=(dc == 0), stop=(dc == DC - 1),
                            )
                        nc.scalar.activation(ptb[:, S0IDX[pos], :], ps2[:],
                                             Exp, scale=INV_SQRT_D)

                for j in range(8):
                    nc.vector.tensor_mul(ptb[:, j, :], ptb[:, j, :],
                                         mkb[:, j, :])
                for pos in range(SB):
                    nc.vector.tensor_mul(pt8[:, pos, :], pt8[:, pos, :],
                                         mk8[:, pos, :])

                # --- slot 0 (bf16): rowsum, AV, normalize, out rows 0..511
                for qs in range(4):
                    kpos = [0, 1, 2, 3] + list(range(8, 9 + qs))
                    idx = [S0IDX[p] for p in kpos]
                    plt = ps_all.tile([P, 512], F32, tag="ps", name="pl")
                    pl = plt[:, :1]
                    for i, j in enumerate(idx):
                        nc.tensor.matmul(
                            pl[:], lhsT=ptb[:, j, ts(qs, P)], rhs=onesb[:],
                            start=(i == 0), stop=(i == len(idx) - 1),
                        )
                    rl = sc_pool.tile([P, 1], F32, tag="rl")
                    nc.vector.reciprocal(rl[:], pl[:])
                    for n in range(4):
                        pav = ps_all.tile([P, 512], F32, tag="ps", name="pav")
                        for i, j in enumerate(idx):
                            nc.tensor.matmul(
                                pav[:], lhsT=ptb[:, j, ts(qs, P)],
                                rhs=vbb[:, j, ts(n, 512)],
                                start=(i == 0), stop=(i == len(idx) - 1),
                            )
                        ob = ob_pool.tile([P, 512], F32, tag="ob")
                        if n % 2 == 0:
                            nc.vector.tensor_scalar_mul(ob[:], pav[:], rl[:])
                        else:
                            nc.scalar.activation(ob[:], pav[:], Copy,
                                                 scale=rl[:])
                        nc.sync.dma_start(
                            out_d.ap()[ds(qs * P, P), ts(n, 512)], ob[:])

                # --- slot 1 (fp8 DR): rows 512..1023
                for qs in range(4):
                    run1 = 6 if qs < 2 else 8   # [0..4+qs] rounded to even
                    pstarts = list(range(0, run1, 2)) + [8, 10, 12, 14]
                    plt = ps_all.tile([P, 512], F32, tag="ps", name="pl8")
                    pl = plt[:, :1]
                    for i, p0 in enumerate(pstarts):
                        nc.tensor.matmul(
                            pl[:], lhsT=pt8[:, ds(p0, 2), ts(qs, P)],
                            rhs=ones8[:],
                            start=(i == 0), stop=(i == len(pstarts) - 1),
                            perf_mode=DR,
                        )
                    rl = sc_pool.tile([P, 1], F32, tag="rl")
                    nc.vector.reciprocal(rl[:], pl[:])
                    for n in range(4):
                        pav = ps_all.tile([P, 512], F32, tag="ps", name="pav8")
                        for i, p0 in enumerate(pstarts):
                            nc.tensor.matmul(
                                pav[:], lhsT=pt8[:, ds(p0, 2), ts(qs, P)],
                                rhs=v8b[:, ds(p0, 2), ts(n, 512)],
                                start=(i == 0), stop=(i == len(pstarts) - 1),
                                perf_mode=DR,
                            )
                        ob = ob_pool.tile([P, 512], F32, tag="ob")
                        if n % 2 == 0:
                            nc.vector.tensor_scalar_mul(ob[:], pav[:], rl[:])
                        else:
                            nc.scalar.activation(ob[:], pav[:], Copy,
                                                 scale=rl[:])
                        nc.sync.dma_start(
                            out_d.ap()[ds(512 + qs * P, P), ts(n, 512)], ob[:])

    nc.compile()
    _CACHED_NC = nc
    return nc


def _host_prep(x, Wq, Wk, Wv):
    """Build per-core input maps (host-side layout prep)."""
    f8 = ml_dtypes.float8_e4m3
    bf = ml_dtypes.bfloat16

    def wqk_layout(W, dt, scale=1.0):
        return np.ascontiguousarray(
            (W * scale).reshape(DC, P, DC, P).transpose(2, 1, 0, 3)).astype(dt)

    def wv_layout(W, dt, scale=1.0):
        return np.ascontiguousarray(
            (W * scale).reshape(2, 8, P, 4, 512).transpose(3, 0, 2, 1, 4)
        ).astype(dt)

    wqb_h = wqk_layout(Wq, bf)
    wq8_h = wqk_layout(Wq, f8, WS)
    wkb_h = wqk_layout(Wk, bf)
    wk8_h = wqk_layout(Wk, f8, WS)
    wvb_h = wv_layout(Wv, bf)
    wv8_h = wv_layout(Wv, f8, WS)

    k_in_block = np.arange(P, dtype=np.int64)[:, None]           # [P, 1]
    q_in_chunk = np.arange(512, dtype=np.int64)[None, :]         # [1, 512]

    def build_masks(h):
        c_lo, c_hi = h, 3 - h
        mb = np.zeros((P, 8, 512), dtype=bf)
        for j, pos in enumerate(SLOT0_POS):
            tkb = POS2TRUE[pos // 4] * 4 + pos % 4
            mb[:, j, :] = (tkb * P + k_in_block) <= (c_lo * 512 + q_in_chunk)
        m8 = np.zeros((P, 16, 512), dtype=f8)
        for pos in range(SB):
            tkb = POS2TRUE[pos // 4] * 4 + pos % 4
            m8[:, pos, :] = (tkb * P + k_in_block) <= (c_hi * 512 + q_in_chunk)
        return mb, m8

    mask_h = [build_masks(0), build_masks(1)]

    in_maps = []
    for core in range(NCORES):
        b, h = divmod(core, 2)
        c_lo, c_hi = h, 3 - h
        xt = x[b].T                                               # [D, S] view
        xtb_h = np.ascontiguousarray(
            xt[:, c_lo * 512:(c_lo + 1) * 512].reshape(DC, P, 512)
            .transpose(1, 0, 2)).astype(bf)
        xt8_h = np.ascontiguousarray(
            xt[:, c_hi * 512:(c_hi + 1) * 512].reshape(DC, P, 512)
            .transpose(1, 0, 2)).astype(f8)
        in_maps.append({
            "xtb": xtb_h,
            "xt8": xt8_h,
            "wqb": wqb_h, "wq8": wq8_h, "wkb": wkb_h, "wk8": wk8_h,
            "wvb": wvb_h, "wv8": wv8_h,
            "maskb": mask_h[h][0], "mask8": mask_h[h][1],
        })
    return in_maps


def run(x, Wq, Wk, Wv, trace=False):
    x = np.asarray(x, dtype=np.float32)
    Wq = np.asarray(Wq, dtype=np.float32)
    Wk = np.asarray(Wk, dtype=np.float32)
    Wv = np.asarray(Wv, dtype=np.float32)
    nc = build_nc()
    in_maps = _host_prep(x, Wq, Wk, Wv)
    res = run_bass_kernel_spmd(nc, in_maps, core_ids=list(range(NCORES)),
                               trace=trace)
    out = np.empty((B, S, D), dtype=np.float32)
    for core in range(NCORES):
        b, h = divmod(core, 2)
        c_lo, c_hi = h, 3 - h
        o = res.results[core]["out"]
        out[b, c_lo * 512:(c_lo + 1) * 512] = o[:512]
        out[b, c_hi * 512:(c_hi + 1) * 512] = o[512:]
    return out, res


def kernel(x, Wq, Wk, Wv):
    out, _ = run(x, Wq, Wk, Wv)
    return out


if __name__ == "__main__":
    build_nc()
    print("build + compile OK")
